# revision 33
# baseline (speedup 1.0000x reference)
"""Two-layer single-head GAT (PyG GATConv semantics) on 8 Trainium2 NeuronCores.

Strategy (dst-sharded edge-parallel, dma_gather-based):
  * Host: add self-loops, sort edges by destination, shard destinations
    across the 8 cores (12500 each).  Segments (per-dst edge runs) are packed
    into GROUPS of <= 64 consecutive segments such that the group's edges
    from each 32768-row source-table window ("chunk") number <= 128.  Each
    group occupies 4 chunk-pure columns of 128 edge slots (one per chunk);
    8 groups form a super-tile (512 psum slots); 4 super-tiles form a
    mega-group whose per-edge source rows are fetched with FOUR
    InstDMAGatherAnt calls (one per table window, int16 indices, 4096 rows
    each) and whose per-edge destination a_dst scalars are fetched with ONE
    more (elem_size=1, indices are shard-local).  This replaces thousands of
    one-index-per-partition SWDGE indirect DMAs (the 10.7ms baseline
    bottleneck: ~1us Q7 descriptor-emission fixed cost per call).
  * Tables are padded to power-of-two row strides (128/64 f32) so row
    addresses encode in dma_gather's stride_bytes_256.  Table rows are
    [feat | 1 | a_src | a_dst | 0...]; gathering feat+2 elements brings the
    constant-1 column that makes the aggregation matmul emit the softmax
    denominator directly.
  * Per super-tile: attention logits e = leakyrelu(a_s + a_d) and p=exp(e)
    in 4 whole-super-tile ops; the exp weights fold into the segment one-hot
    (built in 2 broadcast tensor_tensor ops); 32 matmuls accumulate
    psA[rw, 512] (4 chunk-columns per group); normalisation (+relu+W2 for
    layer 1), transposes, and ONE dma_scatter_add write the per-destination
    rows into the pre-zeroed padded output table (pad slots hit a dump row).
  * AllGather replicates the layer tables between phases.
"""

import numpy as np

N_NODES = 100000
N_CORES = 8
F_IN, H, C = 128, 64, 40

CHUNK = 32768          # dma_gather int16 index window (table rows per window)
NCHUNK = 4             # ceil(100000 / 32768)
GCOL = 128             # edge slots per (group, chunk) column
SEGCAP = 64            # segments per group
SUP = 8                # groups per super-tile  -> 512 psum slots
MEGA = 4               # super-tiles per gather mega-group
DUMMY_SEG = 99.0
W1ROW = 128            # padded layer-1 table row (f32): 512B stride
W2ROW = 64             # padded layer-2 table row (f32): 256B stride

DEF_CFG = dict(
    n=N_NODES, nshard=N_NODES // N_CORES, fin=F_IN, h=H, c=C, ntw=500,
)


# ----------------------------------------------------------------- host prep
def _wrap16(entries):
    """[n] -> [128, n/16] int16: entry i at [i%16, i//16], replicated x8."""
    n = entries.shape[0]
    assert n % 16 == 0
    w = entries.reshape(n // 16, 16).T.astype(np.int16)  # [16, n/16]
    return np.tile(w, (8, 1))


def _pack_core(src_c, dst_c, base, nshard):
    """Group dst-sorted edges: per group, <=SEGCAP consecutive segments with
    <=GCOL edges per source chunk.  Returns per-group data."""
    counts = np.bincount(dst_c - base, minlength=nshard)
    assert counts.min() >= 1
    cum = np.concatenate([[0], np.cumsum(counts)])
    chunk_of = (src_c // CHUNK).astype(np.int64)
    # per-segment chunk counts [nshard, 4]
    segck = np.zeros((nshard, NCHUNK), np.int64)
    for k in range(NCHUNK):
        np.add.at(segck[:, k], dst_c[chunk_of == k] - base, 1)
    assert segck.max() <= GCOL, "single segment overflows a chunk column"
    groups = []
    i = 0
    while i < nshard:
        acc = np.zeros(NCHUNK, np.int64)
        j = i
        while j < nshard and j - i < SEGCAP and (acc + segck[j]).max() <= GCOL:
            acc += segck[j]
            j += 1
        groups.append((i, j))
        i = j
    return groups, cum, chunk_of


def preprocess(edge_index, cfg):
    n, nshard = cfg["n"], cfg["nshard"]
    src = np.asarray(edge_index[0]).astype(np.int64)
    dst = np.asarray(edge_index[1]).astype(np.int64)
    loop = np.arange(n, dtype=np.int64)
    src = np.concatenate([src, loop])
    dst = np.concatenate([dst, loop])
    order = np.argsort(dst, kind="stable")
    src, dst = src[order], dst[order]
    bounds = np.searchsorted(dst, np.arange(N_CORES + 1) * nshard)
    packed = []
    ngmax = 0
    for cc in range(N_CORES):
        s, d = src[bounds[cc]:bounds[cc + 1]], dst[bounds[cc]:bounds[cc + 1]]
        groups, cum, chunk_of = _pack_core(s, d, cc * nshard, nshard)
        packed.append((s, d, groups, cum, chunk_of))
        ngmax = max(ngmax, len(groups))
    nsup = -(-ngmax // SUP)
    nmega = -(-nsup // MEGA)
    nsup = nmega * MEGA
    ng = nsup * SUP
    ncol = nsup * 32  # columns per layer (kappa*8+g per super-tile)

    cores = []
    for cc in range(N_CORES):
        s, d, groups, cum, chunk_of = packed[cc]
        base = cc * nshard
        # per-column edge lists
        rowsidx = np.zeros((nmega, NCHUNK, MEGA, SUP, GCOL), np.int64)
        segid = np.full((128, ncol), DUMMY_SEG, np.float32)
        scat = np.full((nsup, SUP * SEGCAP), nshard, np.int64)
        for gi in range(len(groups)):
            i, j = groups[gi]
            S, g = gi // SUP, gi % SUP
            m, S_sub = S // MEGA, S % MEGA
            e0, e1 = int(cum[i]), int(cum[j])
            ck = chunk_of[e0:e1]
            sg = s[e0:e1]
            dg = d[e0:e1]
            for k in range(NCHUNK):
                sel = np.where(ck == k)[0]
                cnt = sel.shape[0]
                assert cnt <= GCOL
                rowsidx[m, k, S_sub, g, :cnt] = sg[sel] - k * CHUNK
                col = S * 32 + k * SUP + g
                segid[:cnt, col] = (dg[sel] - base - i).astype(np.float32)
            scat[S, g * SEGCAP: g * SEGCAP + (j - i)] = np.arange(i, j)
        adsl = np.minimum(scat, nshard - 1)
        cores.append(dict(
            rowsidx=_wrap16(rowsidx.reshape(-1)).reshape(128, -1),
            segid=segid,
            scat16=np.concatenate(
                [_wrap16(scat[S]) for S in range(nsup)], axis=1),
            adsl16=np.concatenate(
                [_wrap16(adsl[S]) for S in range(nsup)], axis=1),
        ))
    return cores, nsup


def _compress_deps(nc):
    """Drop redundant sync dependencies so walrus' per-instruction HW wait
    slots don't overflow.  Producers on the same engine execute in issue
    order, and DMAs on the same logical queue complete in FIFO order, so a
    dependency on the latest producer of each stream subsumes the earlier
    ones.  Collectives are never dropped."""
    f = nc.m.functions[0]

    def all_insts(blk):
        for i in blk.instructions:
            yield i
        for sb in getattr(blk, "blocks", []) or []:
            yield from all_insts(sb)

    insts = [i for b in f.blocks for i in all_insts(b)]
    pos = {i.name: p for p, i in enumerate(insts)}
    by_name = {i.name: i for i in insts}

    def stream_key(p):
        tname = type(p).__name__
        if tname == "InstCollectiveCompute":
            return None  # own completion semaphore; never compress
        # completion order is FIFO only within one scheduled proc lane
        # (engine proc, or a DMAHW/DMASW semaphore lane)
        proc = getattr(p, "bass_scheduled_proc", None)
        if proc is None:
            return None
        return ("proc", proc)

    for i in insts:
        deps = list(i.sync_dependency_names())
        if len(deps) <= 2:
            continue
        best: dict = {}
        keep = []
        for d in deps:
            p = by_name.get(d)
            if p is None:
                keep.append(d)
                continue
            k = stream_key(p)
            if k is None:
                keep.append(d)
                continue
            cur = best.get(k)
            if cur is None or pos[d] > pos[cur]:
                best[k] = d
        keep += list(best.values())
        for d in deps:
            if d not in keep:
                i.try_remove_dependency(d)


# ------------------------------------------------------------- device program
def build_program(cfg, nsup, debug=False):
    import concourse.bass as bass
    import concourse.bacc as bacc
    import concourse.mybir as mybir
    import concourse.tile as tile
    from concourse import library_config
    from concourse.masks import make_identity

    f32 = mybir.dt.float32
    i16 = mybir.dt.int16
    nshard, fin, h, c = cfg["nshard"], cfg["fin"], cfg["h"], cfg["c"]
    ntw = cfg["ntw"]
    n = cfg["n"]
    nmega = nsup // MEGA
    ncol = nsup * 32
    qw = ntw // 4
    npad = nshard + 128  # padded tables: dump rows at [nshard, npad)

    nc = bacc.Bacc(
        "TRN2", target_bir_lowering=False, debug=False,
        enable_asserts=False, num_devices=N_CORES,
        dynamic_dma_scratch_size=36864,
    )

    xT = nc.dram_tensor("xT", [fin, nshard], f32, kind="ExternalInput").ap()
    w1aug = nc.dram_tensor("w1aug", [fin, W1ROW], f32, kind="ExternalInput").ap()
    w2aug = nc.dram_tensor("w2aug", [h, W2ROW], f32, kind="ExternalInput").ap()
    b1 = nc.dram_tensor("b1", [h, 1], f32, kind="ExternalInput").ap()
    b2rep = nc.dram_tensor("b2rep", [128, c], f32, kind="ExternalInput").ap()
    e1 = nc.dram_tensor("e1", [W1ROW, 1], f32, kind="ExternalInput").ap()
    e2 = nc.dram_tensor("e2", [W2ROW, 1], f32, kind="ExternalInput").ap()
    rowsidx = nc.dram_tensor("rowsidx", [128, nmega * 1024], i16,
                             kind="ExternalInput").ap()
    segid = nc.dram_tensor("segid", [128, ncol], f32, kind="ExternalInput").ap()
    scat16 = nc.dram_tensor("scat16", [128, nsup * 32], i16,
                            kind="ExternalInput").ap()
    adsl16 = nc.dram_tensor("adsl16", [128, nsup * 32], i16,
                            kind="ExternalInput").ap()
    out2 = nc.dram_tensor("out2", [npad, W2ROW], f32, kind="ExternalOutput").ap()

    def raw_dma_gather(out_ap, in_ap, idxs_ap, num_idxs, elem_size, sb256):
        # <=1024 idxs per call: 65 descriptors fits the SWDGE ring with room
        # to pipeline, and the 64-data-desc packet stays within the SDMA
        # packet limit (single_packet keeps the ~35ns/desc drain rate).
        g = nc.gpsimd
        _in_ap = g.lower_ap_dma(in_ap, for_custom_bir_dma=True)
        _idxs_ap = g.lower_ap(idxs_ap)
        _out_ap = g.lower_ap(out_ap)
        return g.add_instruction(
            mybir.InstDMAGatherAnt(
                name=nc.get_next_instruction_name(),
                ins=[*_in_ap, _idxs_ap, g.lower_val_access(g.to_reg(num_idxs))],
                outs=[_out_ap],
                transpose=False, num_idxs=num_idxs, elem_size=elem_size,
                stride_bytes_256=sb256, gen_mode=0, single_packet=True,
                queue_num=0, sbuf_tokens_per_rank=0, sbuf_free_dim_per_rank=0,
                sbuf_free_dim_pad_per_rank=0, sbuf_byte_offset=0,
            )
        )

    with tile.TileContext(nc) as tc:
        with (
            tc.tile_pool(name="consts", bufs=1) as cpool,
            tc.tile_pool(name="mega", bufs=2) as rpool,
            tc.tile_pool(name="work", bufs=2) as wpool,
            tc.tile_pool(name="epil", bufs=2) as epool,
            tc.tile_pool(name="psum", bufs=2, space="PSUM") as pp,
            tc.tile_pool(name="dram", bufs=1, space="DRAM") as dpool,
        ):
            nc.gpsimd.load_library(library_config.mlp)
            # ---- constants
            w1aug_sb = cpool.tile([fin, W1ROW], f32, name="w1aug_sb")
            nc.sync.dma_start(w1aug_sb[:], w1aug)
            w2aug_sb = cpool.tile([h, W2ROW], f32, name="w2aug_sb")
            nc.sync.dma_start(w2aug_sb[:], w2aug)
            b1_sb = cpool.tile([h, 1], f32, name="b1_sb")
            nc.sync.dma_start(b1_sb[:], b1)
            b2rep_sb = cpool.tile([128, c], f32, name="b2rep_sb")
            nc.sync.dma_start(b2rep_sb[:], b2rep)
            e1_sb = cpool.tile([W1ROW, 1], f32, name="e1_sb")
            nc.sync.dma_start(e1_sb[:], e1)
            e2_sb = cpool.tile([W2ROW, 1], f32, name="e2_sb")
            nc.sync.dma_start(e2_sb[:], e2)
            ident = cpool.tile([128, 128], f32, name="ident")
            make_identity(nc, ident[:])
            ones_sb = cpool.tile([1, h], f32, name="ones_sb")
            nc.vector.memset(ones_sb[:], 1.0)
            iota_i = cpool.tile([128, SEGCAP], mybir.dt.int32, name="iota_i")
            nc.gpsimd.iota(iota_i[:], pattern=[[1, SEGCAP]], base=0,
                           channel_multiplier=0)
            iota_f = cpool.tile([128, SEGCAP], f32, name="iota_f")
            nc.vector.tensor_copy(iota_f[:], iota_i[:])
            segid_sb = cpool.tile([128, ncol], f32, name="segid_sb")
            nc.sync.dma_start(segid_sb[:], segid)
            scat_sb = cpool.tile([128, nsup * 32], i16, name="scat_sb")
            nc.sync.dma_start(scat_sb[:], scat16)
            adsl_sb = cpool.tile([128, nsup * 32], i16, name="adsl_sb")
            nc.sync.dma_start(adsl_sb[:], adsl16)
            onesM = cpool.tile([128, 128], f32, name="onesM")
            nc.vector.memset(onesM[:], 1.0)
            zt = cpool.tile([128, 2048], f32, name="zt")
            nc.vector.memset(zt[:], 0.0)

            # ---- internal DRAM tables
            h1s = dpool.tile([nshard, W1ROW], f32, name="h1s")
            h1f = dpool.tile([n, W1ROW], f32, name="h1f", addr_space="Shared")
            g2s = dpool.tile([npad, W2ROW], f32, name="g2s")
            g2f = dpool.tile([n, W2ROW], f32, name="g2f", addr_space="Shared")

            # zero-fill the scatter-add destinations
            for tbl in (g2s, out2):
                for o in range(0, npad, 4096):
                    nr = min(4096, npad - o)
                    nc.sync.dma_start(tbl[o:o + nr, :],
                                      zt[:, 0:nr * W2ROW // 128])

            # ---- phase 0: h1aug shard = (x @ W1aug) rows for this shard
            for nt in range(nshard // ntw):
                o = nt * ntw
                xt = epool.tile([fin, ntw], f32, name="xt")
                nc.sync.dma_start(xt[:], xT[:, o:o + ntw])
                psH = pp.tile([W1ROW, ntw], f32, name="psH", tag="pA")
                nc.tensor.matmul(psH[:], lhsT=w1aug_sb[:], rhs=xt[:],
                                 start=True, stop=True)
                h1t = epool.tile([W1ROW, ntw], f32, name="h1t")
                nc.scalar.activation(h1t[:], psH[:],
                                     mybir.ActivationFunctionType.Identity,
                                     bias=e1_sb[:])
                psT = pp.tile([qw, 4 * W1ROW], f32, name="psT", tag="pD")
                for q in range(4):
                    nc.tensor.transpose(
                        psT[:, q * W1ROW:(q + 1) * W1ROW],
                        in_=h1t[:, q * qw:(q + 1) * qw],
                        identity=ident[:],
                    )
                h1r = epool.tile([qw, 4 * W1ROW], f32, name="h1r")
                nc.vector.tensor_copy(h1r[:], psT[:])
                for q in range(4):
                    nc.sync.dma_start(
                        h1s[o + q * qw:o + (q + 1) * qw, :],
                        h1r[:, q * W1ROW:(q + 1) * W1ROW],
                    )

            nc.gpsimd.collective_compute(
                "AllGather", mybir.AluOpType.bypass,
                replica_groups=[list(range(N_CORES))],
                ins=[h1s[:]], outs=[h1f[:]],
            )

            # ---- edge phases
            def edge_layer(table, trow, adcol, adsb, fdim, out_dram, last):
                rw = fdim + 2
                tb256 = trow * 4 // 256
                for m in range(nmega):
                    ridx = rpool.tile([128, 1024], i16, name="ridx")
                    nc.sync.dma_start(
                        ridx[:], rowsidx[:, m * 1024:(m + 1) * 1024])
                    rows = rpool.tile([128, 128 * rw], f32, name="rows")
                    for k in range(NCHUNK):
                        lo = k * CHUNK
                        hi = min(lo + CHUNK, n)
                        for hh in range(4):
                            raw_dma_gather(
                                rows[:, (k * 32 + hh * 8) * rw:
                                     (k * 32 + hh * 8 + 8) * rw].rearrange(
                                    "p (b e) -> p b e", e=rw),
                                table[lo:hi, :],
                                ridx[:, k * 256 + hh * 64:
                                     k * 256 + (hh + 1) * 64],
                                1024, rw, tb256)
                    rv = rows[:].rearrange("p (k s g e) -> p k s g e",
                                           k=NCHUNK, s=MEGA, e=rw)
                    for S_sub in range(MEGA):
                        S = m * MEGA + S_sub
                        # per-slot a_dst -> partition-replicated psW
                        adcyc = wpool.tile([128, 4], f32, name="adcyc")
                        raw_dma_gather(
                            adcyc[:].rearrange("p (b e) -> p b e", e=1),
                            adcol, adsl_sb[:, S * 32:(S + 1) * 32],
                            512, 1, adsb)
                        diag = wpool.tile([128, 4 * 128], f32, name="diag")
                        nc.vector.tensor_tensor(
                            out=diag[:].rearrange("p (q f) -> p q f", f=128),
                            in0=ident[:].unsqueeze(1).broadcast_to(
                                [128, 4, 128]),
                            in1=adcyc[:].unsqueeze(2).broadcast_to(
                                [128, 4, 128]),
                            op=mybir.AluOpType.mult)
                        psW = pp.tile([128, SUP * SEGCAP], f32, name="psW",
                                      tag="pB")
                        nc.tensor.matmul(psW[:], lhsT=onesM[:], rhs=diag[:],
                                         start=True, stop=True)
                        # unweighted one-hot
                        ohw = wpool.tile([128, 32 * SEGCAP], f32, name="oh")
                        ov = ohw[:].rearrange("p (c s) -> p c s", s=SEGCAP)
                        nc.vector.tensor_tensor(
                            out=ov,
                            in0=iota_f[:].unsqueeze(1).broadcast_to(
                                [128, 32, SEGCAP]),
                            in1=segid_sb[:, S * 32:(S + 1) * 32]
                                .unsqueeze(2).broadcast_to([128, 32, SEGCAP]),
                            op=mybir.AluOpType.is_equal)
                        # per-edge a_dst = reduce_s(ind * psW[slot])
                        scr = wpool.tile([128, 32 * SEGCAP], f32, name="scr")
                        psw_v = psW[:].rearrange(
                            "p (g s) -> p g s", s=SEGCAP).unsqueeze(
                            1).broadcast_to([128, NCHUNK, SUP, SEGCAP])
                        nc.vector.tensor_tensor(
                            out=scr[:].rearrange("p (k g s) -> p k g s",
                                                 k=NCHUNK, s=SEGCAP),
                            in0=ov.rearrange("p (k g) s -> p k g s",
                                             k=NCHUNK),
                            in1=psw_v,
                            op=mybir.AluOpType.mult)
                        es = wpool.tile([128, 32], f32, name="es")
                        nc.vector.tensor_reduce(
                            es[:].rearrange("p (k g) -> p k g", k=NCHUNK),
                            scr[:].rearrange("p (k g s) -> p k g s",
                                             k=NCHUNK, s=SEGCAP),
                            axis=mybir.AxisListType.X,
                            op=mybir.AluOpType.add)
                        # es += a_s ; lrelu ; exp
                        as_v = rv[:, :, S_sub][:, :, :, rw - 1]  # [128,4,8]
                        nc.vector.tensor_tensor(
                            out=es[:].rearrange("p (k g) -> p k g", k=NCHUNK),
                            in0=es[:].rearrange("p (k g) -> p k g", k=NCHUNK),
                            in1=as_v, op=mybir.AluOpType.add)
                        e2t = wpool.tile([128, 32], f32, name="e2")
                        nc.vector.tensor_scalar_mul(e2t[:], es[:], 0.2)
                        nc.vector.tensor_tensor(out=es[:], in0=es[:],
                                                in1=e2t[:],
                                                op=mybir.AluOpType.max)
                        ps = wpool.tile([128, 32], f32, name="ps")
                        nc.scalar.activation(ps[:], es[:],
                                             mybir.ActivationFunctionType.Exp)
                        nc.vector.tensor_tensor(
                            out=ov, in0=ov,
                            in1=ps[:].unsqueeze(2).broadcast_to(
                                [128, 32, SEGCAP]),
                            op=mybir.AluOpType.mult)
                        psA = pp.tile([rw, SUP * SEGCAP], f32,
                                      name="psA", tag="pA")
                        for g in range(SUP):
                            for k in range(NCHUNK):
                                off = ((k * MEGA + S_sub) * SUP + g) * rw
                                nc.tensor.matmul(
                                    psA[:, g * SEGCAP:(g + 1) * SEGCAP],
                                    lhsT=rows[:, off:off + rw],
                                    rhs=ohw[:, (k * SUP + g) * SEGCAP:
                                            (k * SUP + g + 1) * SEGCAP],
                                    start=(k == 0), stop=(k == NCHUNK - 1))
                        asb = epool.tile([rw, SUP * SEGCAP], f32,
                                         name="asb")
                        nc.vector.tensor_copy(asb[:], psA[:])
                        if not last:
                            denr = epool.tile([1, SUP * SEGCAP], f32,
                                              name="denr")
                            nc.vector.reciprocal(denr[:],
                                                 asb[fdim:fdim + 1, :])
                            psB = pp.tile([fdim, SUP * SEGCAP], f32,
                                          name="psB", tag="pB")
                            nc.tensor.matmul(psB[:], lhsT=ones_sb[:, 0:fdim],
                                             rhs=denr[:], start=True,
                                             stop=True)
                            hn = epool.tile([fdim, SUP * SEGCAP], f32,
                                            name="hn")
                            nc.vector.tensor_tensor(
                                out=hn[:], in0=asb[0:fdim, :], in1=psB[:],
                                op=mybir.AluOpType.mult)
                            h2r = epool.tile([fdim, SUP * SEGCAP], f32,
                                             name="h2r")
                            nc.scalar.activation(
                                h2r[:], hn[:],
                                mybir.ActivationFunctionType.Relu,
                                bias=b1_sb[:])
                            psC = pp.tile([W2ROW, SUP * SEGCAP], f32,
                                          name="psC", tag="pC")
                            nc.tensor.matmul(psC[:], lhsT=w2aug_sb[:],
                                             rhs=h2r[:], start=True, stop=True)
                            fin_t = epool.tile([W2ROW, SUP * SEGCAP], f32,
                                               name="fin1")
                            nc.scalar.activation(
                                fin_t[:], psC[:],
                                mybir.ActivationFunctionType.Identity,
                                bias=e2_sb[:])
                            ow = W2ROW
                            psD = pp.tile([128, 4 * ow], f32, name="psD",
                                          tag="pD")
                            for q in range(4):
                                nc.tensor.transpose(
                                    psD[:, q * ow:(q + 1) * ow],
                                    in_=fin_t[:, q * 128:(q + 1) * 128],
                                    identity=ident[0:ow, 0:ow])
                            orows = epool.tile([128, 4 * ow], f32,
                                               name="orows")
                            nc.vector.tensor_copy(orows[:], psD[:])
                            nc.gpsimd.dma_scatter_add(
                                out_dram[:, 0:ow],
                                orows[:].rearrange("p (q e) -> p q e", e=ow),
                                scat_sb[:, S * 32:(S + 1) * 32],
                                512, 512, ow)
                        else:
                            psD = pp.tile([128, 4 * rw], f32, name="psD",
                                          tag="pD")
                            for q in range(4):
                                nc.tensor.transpose(
                                    psD[:, q * rw:(q + 1) * rw],
                                    in_=asb[:, q * 128:(q + 1) * 128],
                                    identity=ident[0:rw, 0:rw])
                            oru = epool.tile([128, 4 * rw], f32, name="oru")
                            nc.vector.tensor_copy(oru[:], psD[:])
                            ouv = oru[:].rearrange("p (q e) -> p q e", e=rw)
                            rec = epool.tile([128, 4], f32, name="rec")
                            nc.vector.reciprocal(rec[:], ouv[:, :, fdim])
                            orows = epool.tile([128, 4 * c], f32,
                                               name="orows")
                            for q in range(4):
                                nc.vector.tensor_scalar_mul(
                                    orows[:, q * c:(q + 1) * c],
                                    oru[:, q * rw:q * rw + c],
                                    rec[:, q:q + 1])
                            nc.vector.tensor_tensor(
                                out=orows[:].rearrange(
                                    "p (q e) -> p q e", e=c),
                                in0=orows[:].rearrange(
                                    "p (q e) -> p q e", e=c),
                                in1=b2rep_sb[:].unsqueeze(1).broadcast_to(
                                    [128, 4, c]),
                                op=mybir.AluOpType.add)
                            nc.gpsimd.dma_scatter_add(
                                out_dram[:, 0:c],
                                orows[:].rearrange("p (q e) -> p q e", e=c),
                                scat_sb[:, S * 32:(S + 1) * 32],
                                512, 512, c, elem_step=W2ROW)

            edge_layer(h1f, W1ROW, h1s[:, h + 2:h + 3], 2, h, g2s, last=False)
            nc.gpsimd.collective_compute(
                "AllGather", mybir.AluOpType.bypass,
                replica_groups=[list(range(N_CORES))],
                ins=[g2s[0:nshard, :]], outs=[g2f[:]],
            )
            edge_layer(g2f, W2ROW, g2s[:, c + 2:c + 3], 1, c, out2, last=True)

    _compress_deps(nc)
    nc.compile()
    return nc


# ------------------------------------------------------------------ interface
def make_inmaps(inputs, cfg):
    x = np.ascontiguousarray(np.asarray(inputs["x"], np.float32))
    W1 = np.asarray(inputs["W1"], np.float32)
    as1 = np.asarray(inputs["att_src1"], np.float32)
    ad1 = np.asarray(inputs["att_dst1"], np.float32)
    b1 = np.asarray(inputs["b1"], np.float32)
    W2 = np.asarray(inputs["W2"], np.float32)
    as2 = np.asarray(inputs["att_src2"], np.float32)
    ad2 = np.asarray(inputs["att_dst2"], np.float32)
    b2 = np.asarray(inputs["b2"], np.float32)
    cores, nsup = preprocess(np.asarray(inputs["edge_index"]), cfg)
    h, cdim, fin = cfg["h"], cfg["c"], cfg["fin"]
    w1aug = np.zeros((fin, W1ROW), np.float32)
    w1aug[:, 0:h] = W1
    w1aug[:, h + 1] = W1 @ as1
    w1aug[:, h + 2] = W1 @ ad1
    w2aug = np.zeros((h, W2ROW), np.float32)
    w2aug[:, 0:cdim] = W2
    w2aug[:, cdim + 1] = W2 @ as2
    w2aug[:, cdim + 2] = W2 @ ad2
    e1v = np.zeros((W1ROW, 1), np.float32)
    e1v[h, 0] = 1.0
    e2v = np.zeros((W2ROW, 1), np.float32)
    e2v[cdim, 0] = 1.0
    nshard = cfg["nshard"]
    in_maps = []
    for cidx in range(N_CORES):
        xs = x[cidx * nshard:(cidx + 1) * nshard]
        in_maps.append(dict(
            xT=np.ascontiguousarray(xs.T),
            w1aug=w1aug, w2aug=w2aug,
            b1=np.ascontiguousarray(b1[:, None]),
            b2rep=np.ascontiguousarray(np.tile(b2[None, :], (128, 1))),
            e1=e1v, e2=e2v,
            rowsidx=cores[cidx]["rowsidx"],
            segid=cores[cidx]["segid"],
            scat16=cores[cidx]["scat16"],
            adsl16=cores[cidx]["adsl16"],
        ))
    return in_maps, nsup


def kernel(**inputs):
    from concourse import bass_utils

    cfg = dict(DEF_CFG)
    in_maps, nsup = make_inmaps(inputs, cfg)
    nc = build_program(cfg, nsup)
    res = bass_utils.run_bass_kernel_spmd(
        nc, in_maps, core_ids=list(range(N_CORES)))
    nshard = cfg["nshard"]
    out = np.concatenate(
        [res.results[c]["out2"][:nshard, :cfg["c"]] for c in range(N_CORES)], 0)
    return out.astype(np.float32)


# revision 37
# speedup vs baseline: 1.1106x; 1.1106x over previous
"""Two-layer single-head GAT (PyG GATConv semantics) on 8 Trainium2 NeuronCores.

Strategy (dst-sharded edge-parallel, dma_gather-based):
  * Host: add self-loops, sort edges by destination, shard destinations
    across the 8 cores (12500 each).  Segments (per-dst edge runs) are packed
    into GROUPS of <= 64 consecutive segments such that the group's edges
    from each 32768-row source-table window ("chunk") number <= 128.  Each
    group occupies 4 chunk-pure columns of 128 edge slots (one per chunk);
    8 groups form a super-tile (512 psum slots); 4 super-tiles form a
    mega-group whose per-edge source rows are fetched with FOUR
    InstDMAGatherAnt calls (one per table window, int16 indices, 4096 rows
    each) and whose per-edge destination a_dst scalars are fetched with ONE
    more (elem_size=1, indices are shard-local).  This replaces thousands of
    one-index-per-partition SWDGE indirect DMAs (the 10.7ms baseline
    bottleneck: ~1us Q7 descriptor-emission fixed cost per call).
  * Tables are padded to power-of-two row strides (128/64 f32) so row
    addresses encode in dma_gather's stride_bytes_256.  Table rows are
    [feat | 1 | a_src | a_dst | 0...]; gathering feat+2 elements brings the
    constant-1 column that makes the aggregation matmul emit the softmax
    denominator directly.
  * Per super-tile: attention logits e = leakyrelu(a_s + a_d) and p=exp(e)
    in 4 whole-super-tile ops; the exp weights fold into the segment one-hot
    (built in 2 broadcast tensor_tensor ops); 32 matmuls accumulate
    psA[rw, 512] (4 chunk-columns per group); normalisation (+relu+W2 for
    layer 1), transposes, and ONE dma_scatter_add write the per-destination
    rows into the pre-zeroed padded output table (pad slots hit a dump row).
  * AllGather replicates the layer tables between phases.
"""

import numpy as np

N_NODES = 100000
N_CORES = 8
F_IN, H, C = 128, 64, 40

CHUNK = 32768          # dma_gather int16 index window (table rows per window)
NCHUNK = 4             # ceil(100000 / 32768)
GCOL = 128             # edge slots per (group, chunk) column
SEGCAP = 64            # segments per group
SUP = 8                # groups per super-tile  -> 512 psum slots
MEGA = 4               # super-tiles per gather mega-group
DUMMY_SEG = 99.0
W1ROW = 128            # padded layer-1 table row (f32): 512B stride
W2ROW = 64             # padded layer-2 table row (f32): 256B stride

DEF_CFG = dict(
    n=N_NODES, nshard=N_NODES // N_CORES, fin=F_IN, h=H, c=C, ntw=500,
)


# ----------------------------------------------------------------- host prep
def _wrap16(entries):
    """[n] -> [128, n/16] int16: entry i at [i%16, i//16], replicated x8."""
    n = entries.shape[0]
    assert n % 16 == 0
    w = entries.reshape(n // 16, 16).T.astype(np.int16)  # [16, n/16]
    return np.tile(w, (8, 1))


def _pack_core(src_c, dst_c, base, nshard):
    """Group dst-sorted edges: per group, <=SEGCAP consecutive segments with
    <=GCOL edges per source chunk.  Returns per-group data."""
    counts = np.bincount(dst_c - base, minlength=nshard)
    assert counts.min() >= 1
    cum = np.concatenate([[0], np.cumsum(counts)])
    chunk_of = (src_c // CHUNK).astype(np.int64)
    # per-segment chunk counts [nshard, 4]
    segck = np.zeros((nshard, NCHUNK), np.int64)
    for k in range(NCHUNK):
        np.add.at(segck[:, k], dst_c[chunk_of == k] - base, 1)
    assert segck.max() <= GCOL, "single segment overflows a chunk column"
    groups = []
    i = 0
    while i < nshard:
        acc = np.zeros(NCHUNK, np.int64)
        j = i
        while j < nshard and j - i < SEGCAP and (acc + segck[j]).max() <= GCOL:
            acc += segck[j]
            j += 1
        groups.append((i, j))
        i = j
    return groups, cum, chunk_of


def preprocess(edge_index, cfg):
    n, nshard = cfg["n"], cfg["nshard"]
    src = np.asarray(edge_index[0]).astype(np.int64)
    dst = np.asarray(edge_index[1]).astype(np.int64)
    loop = np.arange(n, dtype=np.int64)
    src = np.concatenate([src, loop])
    dst = np.concatenate([dst, loop])
    order = np.argsort(dst, kind="stable")
    src, dst = src[order], dst[order]
    bounds = np.searchsorted(dst, np.arange(N_CORES + 1) * nshard)
    packed = []
    ngmax = 0
    for cc in range(N_CORES):
        s, d = src[bounds[cc]:bounds[cc + 1]], dst[bounds[cc]:bounds[cc + 1]]
        groups, cum, chunk_of = _pack_core(s, d, cc * nshard, nshard)
        packed.append((s, d, groups, cum, chunk_of))
        ngmax = max(ngmax, len(groups))
    nsup = -(-ngmax // SUP)
    nmega = -(-nsup // MEGA)
    nsup = nmega * MEGA
    ng = nsup * SUP
    ncol = nsup * 32  # columns per layer (kappa*8+g per super-tile)

    cores = []
    for cc in range(N_CORES):
        s, d, groups, cum, chunk_of = packed[cc]
        base = cc * nshard
        # per-column edge lists
        rowsidx = np.zeros((nmega, NCHUNK, MEGA, SUP, GCOL), np.int64)
        segid = np.full((128, ncol), DUMMY_SEG, np.float32)
        scat = np.full((nsup, SUP * SEGCAP), nshard, np.int64)
        for gi in range(len(groups)):
            i, j = groups[gi]
            S, g = gi // SUP, gi % SUP
            m, S_sub = S // MEGA, S % MEGA
            e0, e1 = int(cum[i]), int(cum[j])
            ck = chunk_of[e0:e1]
            sg = s[e0:e1]
            dg = d[e0:e1]
            for k in range(NCHUNK):
                sel = np.where(ck == k)[0]
                cnt = sel.shape[0]
                assert cnt <= GCOL
                rowsidx[m, k, S_sub, g, :cnt] = sg[sel] - k * CHUNK
                col = S * 32 + k * SUP + g
                segid[:cnt, col] = (dg[sel] - base - i).astype(np.float32)
            scat[S, g * SEGCAP: g * SEGCAP + (j - i)] = np.arange(i, j)
        adsl = np.minimum(scat, nshard - 1)
        cores.append(dict(
            rowsidx=_wrap16(rowsidx.reshape(-1)).reshape(128, -1),
            segid=segid,
            scat16=np.concatenate(
                [_wrap16(scat[S]) for S in range(nsup)], axis=1),
            adsl16=np.concatenate(
                [_wrap16(adsl[S]) for S in range(nsup)], axis=1),
        ))
    return cores, nsup


def _compress_deps(nc):
    """Drop redundant sync dependencies so walrus' per-instruction HW wait
    slots don't overflow.  Producers on the same engine execute in issue
    order, and DMAs on the same logical queue complete in FIFO order, so a
    dependency on the latest producer of each stream subsumes the earlier
    ones.  Collectives are never dropped."""
    f = nc.m.functions[0]

    def all_insts(blk):
        for i in blk.instructions:
            yield i
        for sb in getattr(blk, "blocks", []) or []:
            yield from all_insts(sb)

    insts = [i for b in f.blocks for i in all_insts(b)]
    pos = {i.name: p for p, i in enumerate(insts)}
    by_name = {i.name: i for i in insts}

    def stream_key(p):
        tname = type(p).__name__
        if tname == "InstCollectiveCompute":
            return None  # own completion semaphore; never compress
        # completion order is FIFO only within one scheduled proc lane
        # (engine proc, or a DMAHW/DMASW semaphore lane)
        proc = getattr(p, "bass_scheduled_proc", None)
        if proc is None:
            return None
        return ("proc", proc)

    for i in insts:
        deps = list(i.sync_dependency_names())
        if len(deps) <= 2:
            continue
        best: dict = {}
        keep = []
        for d in deps:
            p = by_name.get(d)
            if p is None:
                keep.append(d)
                continue
            k = stream_key(p)
            if k is None:
                keep.append(d)
                continue
            cur = best.get(k)
            if cur is None or pos[d] > pos[cur]:
                best[k] = d
        keep += list(best.values())
        for d in deps:
            if d not in keep:
                i.try_remove_dependency(d)


# ------------------------------------------------------------- device program
def build_program(cfg, nsup, debug=False):
    import concourse.bass as bass
    import concourse.bacc as bacc
    import concourse.mybir as mybir
    import concourse.tile as tile
    from concourse import library_config
    from concourse.masks import make_identity

    f32 = mybir.dt.float32
    i16 = mybir.dt.int16
    nshard, fin, h, c = cfg["nshard"], cfg["fin"], cfg["h"], cfg["c"]
    ntw = cfg["ntw"]
    n = cfg["n"]
    nmega = nsup // MEGA
    ncol = nsup * 32
    qw = ntw // 4
    npad = nshard + 128  # padded tables: dump rows at [nshard, npad)

    nc = bacc.Bacc(
        "TRN2", target_bir_lowering=False, debug=False,
        enable_asserts=False, num_devices=N_CORES,
        dynamic_dma_scratch_size=36864,
    )

    xT = nc.dram_tensor("xT", [fin, nshard], f32, kind="ExternalInput").ap()
    w1aug = nc.dram_tensor("w1aug", [fin, W1ROW], f32, kind="ExternalInput").ap()
    w2aug = nc.dram_tensor("w2aug", [h, W2ROW], f32, kind="ExternalInput").ap()
    b1 = nc.dram_tensor("b1", [h, 1], f32, kind="ExternalInput").ap()
    b2rep = nc.dram_tensor("b2rep", [128, c], f32, kind="ExternalInput").ap()
    e1 = nc.dram_tensor("e1", [W1ROW, 1], f32, kind="ExternalInput").ap()
    e2 = nc.dram_tensor("e2", [W2ROW, 1], f32, kind="ExternalInput").ap()
    rowsidx = nc.dram_tensor("rowsidx", [128, nmega * 1024], i16,
                             kind="ExternalInput").ap()
    segid = nc.dram_tensor("segid", [128, ncol], f32, kind="ExternalInput").ap()
    scat16 = nc.dram_tensor("scat16", [128, nsup * 32], i16,
                            kind="ExternalInput").ap()
    adsl16 = nc.dram_tensor("adsl16", [128, nsup * 32], i16,
                            kind="ExternalInput").ap()
    out2 = nc.dram_tensor("out2", [npad, W2ROW], f32, kind="ExternalOutput").ap()

    def raw_dma_gather(out_ap, in_ap, idxs_ap, num_idxs, elem_size, sb256):
        # <=1024 idxs per call: 65 descriptors fits the SWDGE ring with room
        # to pipeline, and the 64-data-desc packet stays within the SDMA
        # packet limit (single_packet keeps the ~35ns/desc drain rate).
        g = nc.gpsimd
        _in_ap = g.lower_ap_dma(in_ap, for_custom_bir_dma=True)
        _idxs_ap = g.lower_ap(idxs_ap)
        _out_ap = g.lower_ap(out_ap)
        return g.add_instruction(
            mybir.InstDMAGatherAnt(
                name=nc.get_next_instruction_name(),
                ins=[*_in_ap, _idxs_ap, g.lower_val_access(g.to_reg(num_idxs))],
                outs=[_out_ap],
                transpose=False, num_idxs=num_idxs, elem_size=elem_size,
                stride_bytes_256=sb256, gen_mode=0, single_packet=False,
                queue_num=0, sbuf_tokens_per_rank=0, sbuf_free_dim_per_rank=0,
                sbuf_free_dim_pad_per_rank=0, sbuf_byte_offset=0,
            )
        )

    with tile.TileContext(nc) as tc:
        with (
            tc.tile_pool(name="consts", bufs=1) as cpool,
            tc.tile_pool(name="mega", bufs=2) as rpool,
            tc.tile_pool(name="work", bufs=2) as wpool,
            tc.tile_pool(name="epil", bufs=2) as epool,
            tc.tile_pool(name="psum", bufs=2, space="PSUM") as pp,
            tc.tile_pool(name="dram", bufs=1, space="DRAM") as dpool,
        ):
            nc.gpsimd.load_library(library_config.mlp)
            # ---- constants
            w1aug_sb = cpool.tile([fin, W1ROW], f32, name="w1aug_sb")
            nc.sync.dma_start(w1aug_sb[:], w1aug)
            w2aug_sb = cpool.tile([h, W2ROW], f32, name="w2aug_sb")
            nc.sync.dma_start(w2aug_sb[:], w2aug)
            b1_sb = cpool.tile([h, 1], f32, name="b1_sb")
            nc.sync.dma_start(b1_sb[:], b1)
            b2rep_sb = cpool.tile([128, c], f32, name="b2rep_sb")
            nc.sync.dma_start(b2rep_sb[:], b2rep)
            e1_sb = cpool.tile([W1ROW, 1], f32, name="e1_sb")
            nc.sync.dma_start(e1_sb[:], e1)
            e2_sb = cpool.tile([W2ROW, 1], f32, name="e2_sb")
            nc.sync.dma_start(e2_sb[:], e2)
            ident = cpool.tile([128, 128], f32, name="ident")
            make_identity(nc, ident[:])
            ones_sb = cpool.tile([1, h], f32, name="ones_sb")
            nc.vector.memset(ones_sb[:], 1.0)
            iota_i = cpool.tile([128, SEGCAP], mybir.dt.int32, name="iota_i")
            nc.gpsimd.iota(iota_i[:], pattern=[[1, SEGCAP]], base=0,
                           channel_multiplier=0)
            iota_f = cpool.tile([128, SEGCAP], f32, name="iota_f")
            nc.vector.tensor_copy(iota_f[:], iota_i[:])
            segid_sb = cpool.tile([128, ncol], f32, name="segid_sb")
            nc.sync.dma_start(segid_sb[:], segid)
            scat_sb = cpool.tile([128, nsup * 32], i16, name="scat_sb")
            nc.sync.dma_start(scat_sb[:], scat16)
            adsl_sb = cpool.tile([128, nsup * 32], i16, name="adsl_sb")
            nc.sync.dma_start(adsl_sb[:], adsl16)
            onesM = cpool.tile([128, 128], f32, name="onesM")
            nc.vector.memset(onesM[:], 1.0)
            zt = cpool.tile([128, 2048], f32, name="zt")
            nc.vector.memset(zt[:], 0.0)

            # ---- internal DRAM tables
            h1s = dpool.tile([nshard, W1ROW], f32, name="h1s")
            h1f = dpool.tile([n, W1ROW], f32, name="h1f", addr_space="Shared")
            g2s = dpool.tile([npad, W2ROW], f32, name="g2s")
            g2f = dpool.tile([n, W2ROW], f32, name="g2f", addr_space="Shared")

            # zero-fill the scatter-add destinations
            for tbl in (g2s, out2):
                for o in range(0, npad, 4096):
                    nr = min(4096, npad - o)
                    nc.sync.dma_start(tbl[o:o + nr, :],
                                      zt[:, 0:nr * W2ROW // 128])

            # ---- phase 0: h1aug shard = (x @ W1aug) rows for this shard
            for nt in range(nshard // ntw):
                o = nt * ntw
                xt = epool.tile([fin, ntw], f32, name="xt")
                nc.sync.dma_start(xt[:], xT[:, o:o + ntw])
                psH = pp.tile([W1ROW, ntw], f32, name="psH", tag="pA")
                nc.tensor.matmul(psH[:], lhsT=w1aug_sb[:], rhs=xt[:],
                                 start=True, stop=True)
                h1t = epool.tile([W1ROW, ntw], f32, name="h1t")
                nc.scalar.activation(h1t[:], psH[:],
                                     mybir.ActivationFunctionType.Identity,
                                     bias=e1_sb[:])
                psT = pp.tile([qw, 4 * W1ROW], f32, name="psT", tag="pD")
                for q in range(4):
                    nc.tensor.transpose(
                        psT[:, q * W1ROW:(q + 1) * W1ROW],
                        in_=h1t[:, q * qw:(q + 1) * qw],
                        identity=ident[:],
                    )
                h1r = epool.tile([qw, 4 * W1ROW], f32, name="h1r")
                nc.vector.tensor_copy(h1r[:], psT[:])
                for q in range(4):
                    nc.sync.dma_start(
                        h1s[o + q * qw:o + (q + 1) * qw, :],
                        h1r[:, q * W1ROW:(q + 1) * W1ROW],
                    )

            nc.gpsimd.collective_compute(
                "AllGather", mybir.AluOpType.bypass,
                replica_groups=[list(range(N_CORES))],
                ins=[h1s[:]], outs=[h1f[:]],
            )

            # ---- edge phases
            def edge_layer(table, trow, adcol, adsb, fdim, out_dram, last):
                rw = fdim + 2
                tb256 = trow * 4 // 256
                for m in range(nmega):
                    ridx = rpool.tile([128, 1024], i16, name="ridx")
                    nc.sync.dma_start(
                        ridx[:], rowsidx[:, m * 1024:(m + 1) * 1024])
                    rows = rpool.tile([128, 128 * rw], f32, name="rows")
                    for k in range(NCHUNK):
                        lo = k * CHUNK
                        hi = min(lo + CHUNK, n)
                        for hh in range(2):
                            raw_dma_gather(
                                rows[:, (k * 32 + hh * 16) * rw:
                                     (k * 32 + hh * 16 + 16) * rw].rearrange(
                                    "p (b e) -> p b e", e=rw),
                                table[lo:hi, :],
                                ridx[:, k * 256 + hh * 128:
                                     k * 256 + (hh + 1) * 128],
                                2048, rw, tb256)
                    # per-slot a_dst for the whole mega-group in one call
                    adcm = rpool.tile([128, 16], f32, name="adcm")
                    raw_dma_gather(
                        adcm[:].rearrange("p (b e) -> p b e", e=1),
                        adcol, adsl_sb[:, m * 128:(m + 1) * 128],
                        2048, 1, adsb)
                    rv = rows[:].rearrange("p (k s g e) -> p k s g e",
                                           k=NCHUNK, s=MEGA, e=rw)
                    for S_sub in range(MEGA):
                        S = m * MEGA + S_sub
                        adcyc = adcm[:, S_sub * 4:(S_sub + 1) * 4]
                        diag = wpool.tile([128, 4 * 128], f32, name="diag")
                        nc.vector.tensor_tensor(
                            out=diag[:].rearrange("p (q f) -> p q f", f=128),
                            in0=ident[:].unsqueeze(1).broadcast_to(
                                [128, 4, 128]),
                            in1=adcyc.unsqueeze(2).broadcast_to(
                                [128, 4, 128]),
                            op=mybir.AluOpType.mult)
                        psW = pp.tile([128, SUP * SEGCAP], f32, name="psW",
                                      tag="pB")
                        nc.tensor.matmul(psW[:], lhsT=onesM[:], rhs=diag[:],
                                         start=True, stop=True)
                        # unweighted one-hot
                        ohw = wpool.tile([128, 32 * SEGCAP], f32, name="oh")
                        ov = ohw[:].rearrange("p (c s) -> p c s", s=SEGCAP)
                        nc.vector.tensor_tensor(
                            out=ov,
                            in0=iota_f[:].unsqueeze(1).broadcast_to(
                                [128, 32, SEGCAP]),
                            in1=segid_sb[:, S * 32:(S + 1) * 32]
                                .unsqueeze(2).broadcast_to([128, 32, SEGCAP]),
                            op=mybir.AluOpType.is_equal)
                        # per-edge a_dst = reduce_s(ind * psW[slot])
                        scr = wpool.tile([128, 32 * SEGCAP], f32, name="scr")
                        psw_v = psW[:].rearrange(
                            "p (g s) -> p g s", s=SEGCAP).unsqueeze(
                            1).broadcast_to([128, NCHUNK, SUP, SEGCAP])
                        nc.vector.tensor_tensor(
                            out=scr[:].rearrange("p (k g s) -> p k g s",
                                                 k=NCHUNK, s=SEGCAP),
                            in0=ov.rearrange("p (k g) s -> p k g s",
                                             k=NCHUNK),
                            in1=psw_v,
                            op=mybir.AluOpType.mult)
                        es = wpool.tile([128, 32], f32, name="es")
                        nc.vector.tensor_reduce(
                            es[:].rearrange("p (k g) -> p k g", k=NCHUNK),
                            scr[:].rearrange("p (k g s) -> p k g s",
                                             k=NCHUNK, s=SEGCAP),
                            axis=mybir.AxisListType.X,
                            op=mybir.AluOpType.add)
                        # es += a_s ; lrelu ; exp
                        as_v = rv[:, :, S_sub][:, :, :, rw - 1]  # [128,4,8]
                        nc.vector.tensor_tensor(
                            out=es[:].rearrange("p (k g) -> p k g", k=NCHUNK),
                            in0=es[:].rearrange("p (k g) -> p k g", k=NCHUNK),
                            in1=as_v, op=mybir.AluOpType.add)
                        e2t = wpool.tile([128, 32], f32, name="e2")
                        nc.vector.tensor_scalar_mul(e2t[:], es[:], 0.2)
                        nc.vector.tensor_tensor(out=es[:], in0=es[:],
                                                in1=e2t[:],
                                                op=mybir.AluOpType.max)
                        ps = wpool.tile([128, 32], f32, name="ps")
                        nc.scalar.activation(ps[:], es[:],
                                             mybir.ActivationFunctionType.Exp)
                        nc.vector.tensor_tensor(
                            out=ov, in0=ov,
                            in1=ps[:].unsqueeze(2).broadcast_to(
                                [128, 32, SEGCAP]),
                            op=mybir.AluOpType.mult)
                        psA = pp.tile([rw, SUP * SEGCAP], f32,
                                      name="psA", tag="pA")
                        for g in range(SUP):
                            for k in range(NCHUNK):
                                off = ((k * MEGA + S_sub) * SUP + g) * rw
                                nc.tensor.matmul(
                                    psA[:, g * SEGCAP:(g + 1) * SEGCAP],
                                    lhsT=rows[:, off:off + rw],
                                    rhs=ohw[:, (k * SUP + g) * SEGCAP:
                                            (k * SUP + g + 1) * SEGCAP],
                                    start=(k == 0), stop=(k == NCHUNK - 1))
                        asb = epool.tile([rw, SUP * SEGCAP], f32,
                                         name="asb")
                        nc.vector.tensor_copy(asb[:], psA[:])
                        if not last:
                            denr = epool.tile([1, SUP * SEGCAP], f32,
                                              name="denr")
                            nc.vector.reciprocal(denr[:],
                                                 asb[fdim:fdim + 1, :])
                            psB = pp.tile([fdim, SUP * SEGCAP], f32,
                                          name="psB", tag="pB")
                            nc.tensor.matmul(psB[:], lhsT=ones_sb[:, 0:fdim],
                                             rhs=denr[:], start=True,
                                             stop=True)
                            hn = epool.tile([fdim, SUP * SEGCAP], f32,
                                            name="hn")
                            nc.vector.tensor_tensor(
                                out=hn[:], in0=asb[0:fdim, :], in1=psB[:],
                                op=mybir.AluOpType.mult)
                            h2r = epool.tile([fdim, SUP * SEGCAP], f32,
                                             name="h2r")
                            nc.scalar.activation(
                                h2r[:], hn[:],
                                mybir.ActivationFunctionType.Relu,
                                bias=b1_sb[:])
                            psC = pp.tile([W2ROW, SUP * SEGCAP], f32,
                                          name="psC", tag="pC")
                            nc.tensor.matmul(psC[:], lhsT=w2aug_sb[:],
                                             rhs=h2r[:], start=True, stop=True)
                            fin_t = epool.tile([W2ROW, SUP * SEGCAP], f32,
                                               name="fin1")
                            nc.scalar.activation(
                                fin_t[:], psC[:],
                                mybir.ActivationFunctionType.Identity,
                                bias=e2_sb[:])
                            ow = W2ROW
                            psD = pp.tile([128, 4 * ow], f32, name="psD",
                                          tag="pD")
                            for q in range(4):
                                nc.tensor.transpose(
                                    psD[:, q * ow:(q + 1) * ow],
                                    in_=fin_t[:, q * 128:(q + 1) * 128],
                                    identity=ident[0:ow, 0:ow])
                            orows = epool.tile([128, 4 * ow], f32,
                                               name="orows")
                            nc.vector.tensor_copy(orows[:], psD[:])
                            nc.gpsimd.dma_scatter_add(
                                out_dram[:, 0:ow],
                                orows[:].rearrange("p (q e) -> p q e", e=ow),
                                scat_sb[:, S * 32:(S + 1) * 32],
                                512, 512, ow)
                        else:
                            psD = pp.tile([128, 4 * rw], f32, name="psD",
                                          tag="pD")
                            for q in range(4):
                                nc.tensor.transpose(
                                    psD[:, q * rw:(q + 1) * rw],
                                    in_=asb[:, q * 128:(q + 1) * 128],
                                    identity=ident[0:rw, 0:rw])
                            oru = epool.tile([128, 4 * rw], f32, name="oru")
                            nc.vector.tensor_copy(oru[:], psD[:])
                            ouv = oru[:].rearrange("p (q e) -> p q e", e=rw)
                            rec = epool.tile([128, 4], f32, name="rec")
                            nc.vector.reciprocal(rec[:], ouv[:, :, fdim])
                            orows = epool.tile([128, 4 * c], f32,
                                               name="orows")
                            for q in range(4):
                                nc.vector.tensor_scalar_mul(
                                    orows[:, q * c:(q + 1) * c],
                                    oru[:, q * rw:q * rw + c],
                                    rec[:, q:q + 1])
                            nc.vector.tensor_tensor(
                                out=orows[:].rearrange(
                                    "p (q e) -> p q e", e=c),
                                in0=orows[:].rearrange(
                                    "p (q e) -> p q e", e=c),
                                in1=b2rep_sb[:].unsqueeze(1).broadcast_to(
                                    [128, 4, c]),
                                op=mybir.AluOpType.add)
                            nc.gpsimd.dma_scatter_add(
                                out_dram[:, 0:c],
                                orows[:].rearrange("p (q e) -> p q e", e=c),
                                scat_sb[:, S * 32:(S + 1) * 32],
                                512, 512, c, elem_step=W2ROW)

            edge_layer(h1f, W1ROW, h1s[:, h + 2:h + 3], 2, h, g2s, last=False)
            nc.gpsimd.collective_compute(
                "AllGather", mybir.AluOpType.bypass,
                replica_groups=[list(range(N_CORES))],
                ins=[g2s[0:nshard, :]], outs=[g2f[:]],
            )
            edge_layer(g2f, W2ROW, g2s[:, c + 2:c + 3], 1, c, out2, last=True)

    _compress_deps(nc)
    nc.compile()
    return nc


# ------------------------------------------------------------------ interface
def make_inmaps(inputs, cfg):
    x = np.ascontiguousarray(np.asarray(inputs["x"], np.float32))
    W1 = np.asarray(inputs["W1"], np.float32)
    as1 = np.asarray(inputs["att_src1"], np.float32)
    ad1 = np.asarray(inputs["att_dst1"], np.float32)
    b1 = np.asarray(inputs["b1"], np.float32)
    W2 = np.asarray(inputs["W2"], np.float32)
    as2 = np.asarray(inputs["att_src2"], np.float32)
    ad2 = np.asarray(inputs["att_dst2"], np.float32)
    b2 = np.asarray(inputs["b2"], np.float32)
    cores, nsup = preprocess(np.asarray(inputs["edge_index"]), cfg)
    h, cdim, fin = cfg["h"], cfg["c"], cfg["fin"]
    w1aug = np.zeros((fin, W1ROW), np.float32)
    w1aug[:, 0:h] = W1
    w1aug[:, h + 1] = W1 @ as1
    w1aug[:, h + 2] = W1 @ ad1
    w2aug = np.zeros((h, W2ROW), np.float32)
    w2aug[:, 0:cdim] = W2
    w2aug[:, cdim + 1] = W2 @ as2
    w2aug[:, cdim + 2] = W2 @ ad2
    e1v = np.zeros((W1ROW, 1), np.float32)
    e1v[h, 0] = 1.0
    e2v = np.zeros((W2ROW, 1), np.float32)
    e2v[cdim, 0] = 1.0
    nshard = cfg["nshard"]
    in_maps = []
    for cidx in range(N_CORES):
        xs = x[cidx * nshard:(cidx + 1) * nshard]
        in_maps.append(dict(
            xT=np.ascontiguousarray(xs.T),
            w1aug=w1aug, w2aug=w2aug,
            b1=np.ascontiguousarray(b1[:, None]),
            b2rep=np.ascontiguousarray(np.tile(b2[None, :], (128, 1))),
            e1=e1v, e2=e2v,
            rowsidx=cores[cidx]["rowsidx"],
            segid=cores[cidx]["segid"],
            scat16=cores[cidx]["scat16"],
            adsl16=cores[cidx]["adsl16"],
        ))
    return in_maps, nsup


def kernel(**inputs):
    from concourse import bass_utils

    cfg = dict(DEF_CFG)
    in_maps, nsup = make_inmaps(inputs, cfg)
    nc = build_program(cfg, nsup)
    res = bass_utils.run_bass_kernel_spmd(
        nc, in_maps, core_ids=list(range(N_CORES)))
    nshard = cfg["nshard"]
    out = np.concatenate(
        [res.results[c]["out2"][:nshard, :cfg["c"]] for c in range(N_CORES)], 0)
    return out.astype(np.float32)


# revision 41
# speedup vs baseline: 1.1511x; 1.0365x over previous
"""Two-layer single-head GAT (PyG GATConv semantics) on 8 Trainium2 NeuronCores.

Strategy (dst-sharded edge-parallel, dma_gather-based):
  * Host: add self-loops, sort edges by destination, shard destinations
    across the 8 cores (12500 each).  Segments (per-dst edge runs) are packed
    into GROUPS of <= 64 consecutive segments such that the group's edges
    from each 32768-row source-table window ("chunk") number <= 128.  Each
    group occupies 4 chunk-pure columns of 128 edge slots (one per chunk);
    8 groups form a super-tile (512 psum slots); 4 super-tiles form a
    mega-group whose per-edge source rows are fetched with FOUR
    InstDMAGatherAnt calls (one per table window, int16 indices, 4096 rows
    each) and whose per-edge destination a_dst scalars are fetched with ONE
    more (elem_size=1, indices are shard-local).  This replaces thousands of
    one-index-per-partition SWDGE indirect DMAs (the 10.7ms baseline
    bottleneck: ~1us Q7 descriptor-emission fixed cost per call).
  * Tables are padded to power-of-two row strides (128/64 f32) so row
    addresses encode in dma_gather's stride_bytes_256.  Table rows are
    [feat | 1 | a_src | a_dst | 0...]; gathering feat+2 elements brings the
    constant-1 column that makes the aggregation matmul emit the softmax
    denominator directly.
  * Per super-tile: attention logits e = leakyrelu(a_s + a_d) and p=exp(e)
    in 4 whole-super-tile ops; the exp weights fold into the segment one-hot
    (built in 2 broadcast tensor_tensor ops); 32 matmuls accumulate
    psA[rw, 512] (4 chunk-columns per group); normalisation (+relu+W2 for
    layer 1), transposes, and ONE dma_scatter_add write the per-destination
    rows into the pre-zeroed padded output table (pad slots hit a dump row).
  * AllGather replicates the layer tables between phases.
"""

import numpy as np

N_NODES = 100000
N_CORES = 8
F_IN, H, C = 128, 64, 40

CHUNK = 32768          # dma_gather int16 index window (table rows per window)
NCHUNK = 4             # ceil(100000 / 32768)
GCOL = 128             # edge slots per (group, chunk) column
SEGCAP = 64            # segments per group
SUP = 8                # groups per super-tile  -> 512 psum slots
MEGA = 4               # super-tiles per gather mega-group
DUMMY_SEG = 99.0
W1ROW = 128            # padded layer-1 table row (f32): 512B stride
W2ROW = 64             # padded layer-2 table row (f32): 256B stride

DEF_CFG = dict(
    n=N_NODES, nshard=N_NODES // N_CORES, fin=F_IN, h=H, c=C, ntw=500,
)


# ----------------------------------------------------------------- host prep
def _wrap16(entries):
    """[n] -> [128, n/16] int16: entry i at [i%16, i//16], replicated x8."""
    n = entries.shape[0]
    assert n % 16 == 0
    w = entries.reshape(n // 16, 16).T.astype(np.int16)  # [16, n/16]
    return np.tile(w, (8, 1))


def _pack_core(src_c, dst_c, base, nshard):
    """Group dst-sorted edges: per group, <=SEGCAP consecutive segments with
    <=GCOL edges per source chunk.  Returns per-group data."""
    counts = np.bincount(dst_c - base, minlength=nshard)
    assert counts.min() >= 1
    cum = np.concatenate([[0], np.cumsum(counts)])
    chunk_of = (src_c // CHUNK).astype(np.int64)
    # per-segment chunk counts [nshard, 4]
    segck = np.zeros((nshard, NCHUNK), np.int64)
    for k in range(NCHUNK):
        np.add.at(segck[:, k], dst_c[chunk_of == k] - base, 1)
    assert segck.max() <= GCOL, "single segment overflows a chunk column"
    groups = []
    i = 0
    while i < nshard:
        acc = np.zeros(NCHUNK, np.int64)
        j = i
        while j < nshard and j - i < SEGCAP and (acc + segck[j]).max() <= GCOL:
            acc += segck[j]
            j += 1
        groups.append((i, j))
        i = j
    return groups, cum, chunk_of


def preprocess(edge_index, cfg):
    n, nshard = cfg["n"], cfg["nshard"]
    src = np.asarray(edge_index[0]).astype(np.int64)
    dst = np.asarray(edge_index[1]).astype(np.int64)
    loop = np.arange(n, dtype=np.int64)
    src = np.concatenate([src, loop])
    dst = np.concatenate([dst, loop])
    order = np.argsort(dst, kind="stable")
    src, dst = src[order], dst[order]
    bounds = np.searchsorted(dst, np.arange(N_CORES + 1) * nshard)
    packed = []
    ngmax = 0
    for cc in range(N_CORES):
        s, d = src[bounds[cc]:bounds[cc + 1]], dst[bounds[cc]:bounds[cc + 1]]
        groups, cum, chunk_of = _pack_core(s, d, cc * nshard, nshard)
        packed.append((s, d, groups, cum, chunk_of))
        ngmax = max(ngmax, len(groups))
    nsup = -(-ngmax // SUP)
    nmega = -(-nsup // MEGA)
    nsup = nmega * MEGA
    ng = nsup * SUP
    ncol = nsup * 32  # columns per layer (kappa*8+g per super-tile)

    cores = []
    for cc in range(N_CORES):
        s, d, groups, cum, chunk_of = packed[cc]
        base = cc * nshard
        # per-column edge lists
        rowsidx = np.zeros((nmega, NCHUNK, MEGA, SUP, GCOL), np.int64)
        segid = np.full((128, ncol), DUMMY_SEG, np.float32)
        scat = np.full((nsup, SUP * SEGCAP), nshard, np.int64)
        for gi in range(len(groups)):
            i, j = groups[gi]
            S, g = gi // SUP, gi % SUP
            m, S_sub = S // MEGA, S % MEGA
            e0, e1 = int(cum[i]), int(cum[j])
            ck = chunk_of[e0:e1]
            sg = s[e0:e1]
            dg = d[e0:e1]
            for k in range(NCHUNK):
                sel = np.where(ck == k)[0]
                cnt = sel.shape[0]
                assert cnt <= GCOL
                rowsidx[m, k, S_sub, g, :cnt] = sg[sel] - k * CHUNK
                col = S * 32 + k * SUP + g
                segid[:cnt, col] = (dg[sel] - base - i).astype(np.float32)
            scat[S, g * SEGCAP: g * SEGCAP + (j - i)] = np.arange(i, j)
        adsl = np.minimum(scat, nshard - 1)
        cores.append(dict(
            rowsidx=_wrap16(rowsidx.reshape(-1)).reshape(128, -1),
            segid=segid,
            scat16=np.concatenate(
                [_wrap16(scat[S]) for S in range(nsup)], axis=1),
            adsl16=np.concatenate(
                [_wrap16(adsl[S]) for S in range(nsup)], axis=1),
        ))
    return cores, nsup


def _compress_deps(nc):
    """Drop redundant sync dependencies so walrus' per-instruction HW wait
    slots don't overflow.  Producers on the same engine execute in issue
    order, and DMAs on the same logical queue complete in FIFO order, so a
    dependency on the latest producer of each stream subsumes the earlier
    ones.  Collectives are never dropped."""
    f = nc.m.functions[0]

    def all_insts(blk):
        for i in blk.instructions:
            yield i
        for sb in getattr(blk, "blocks", []) or []:
            yield from all_insts(sb)

    insts = [i for b in f.blocks for i in all_insts(b)]
    pos = {i.name: p for p, i in enumerate(insts)}
    by_name = {i.name: i for i in insts}

    def stream_key(p):
        tname = type(p).__name__
        if tname == "InstCollectiveCompute":
            return None  # own completion semaphore; never compress
        # completion order is FIFO only within one scheduled proc lane
        # (engine proc, or a DMAHW/DMASW semaphore lane)
        proc = getattr(p, "bass_scheduled_proc", None)
        if proc is None:
            return None
        return ("proc", proc)

    for i in insts:
        deps = list(i.sync_dependency_names())
        if len(deps) <= 2:
            continue
        best: dict = {}
        keep = []
        for d in deps:
            p = by_name.get(d)
            if p is None:
                keep.append(d)
                continue
            k = stream_key(p)
            if k is None:
                keep.append(d)
                continue
            cur = best.get(k)
            if cur is None or pos[d] > pos[cur]:
                best[k] = d
        keep += list(best.values())
        for d in deps:
            if d not in keep:
                i.try_remove_dependency(d)


# ------------------------------------------------------------- device program
def build_program(cfg, nsup, debug=False):
    import concourse.bass as bass
    import concourse.bacc as bacc
    import concourse.mybir as mybir
    import concourse.tile as tile
    from concourse import library_config
    from concourse.masks import make_identity

    f32 = mybir.dt.float32
    i16 = mybir.dt.int16
    nshard, fin, h, c = cfg["nshard"], cfg["fin"], cfg["h"], cfg["c"]
    ntw = cfg["ntw"]
    n = cfg["n"]
    nmega = nsup // MEGA
    ncol = nsup * 32
    qw = ntw // 4
    npad = nshard + 128  # padded tables: dump rows at [nshard, npad)

    nc = bacc.Bacc(
        "TRN2", target_bir_lowering=False, debug=False,
        enable_asserts=False, num_devices=N_CORES,
        dynamic_dma_scratch_size=36864, num_swdge_queues=2,
    )

    xT = nc.dram_tensor("xT", [fin, nshard], f32, kind="ExternalInput").ap()
    w1aug = nc.dram_tensor("w1aug", [fin, W1ROW], f32, kind="ExternalInput").ap()
    w2aug = nc.dram_tensor("w2aug", [h, W2ROW], f32, kind="ExternalInput").ap()
    b1 = nc.dram_tensor("b1", [h, 1], f32, kind="ExternalInput").ap()
    b2rep = nc.dram_tensor("b2rep", [128, c], f32, kind="ExternalInput").ap()
    e1 = nc.dram_tensor("e1", [W1ROW, 1], f32, kind="ExternalInput").ap()
    e2 = nc.dram_tensor("e2", [W2ROW, 1], f32, kind="ExternalInput").ap()
    rowsidx = nc.dram_tensor("rowsidx", [128, nmega * 1024], i16,
                             kind="ExternalInput").ap()
    segid = nc.dram_tensor("segid", [128, ncol], f32, kind="ExternalInput").ap()
    scat16 = nc.dram_tensor("scat16", [128, nsup * 32], i16,
                            kind="ExternalInput").ap()
    adsl16 = nc.dram_tensor("adsl16", [128, nsup * 32], i16,
                            kind="ExternalInput").ap()
    out2 = nc.dram_tensor("out2", [npad, W2ROW], f32, kind="ExternalOutput").ap()

    def raw_dma_gather(out_ap, in_ap, idxs_ap, num_idxs, elem_size, sb256,
                       qn=0):
        # <=1024 idxs per call: 65 descriptors fits the SWDGE ring with room
        # to pipeline, and the 64-data-desc packet stays within the SDMA
        # packet limit (single_packet keeps the ~35ns/desc drain rate).
        g = nc.gpsimd
        _in_ap = g.lower_ap_dma(in_ap, for_custom_bir_dma=True)
        _idxs_ap = g.lower_ap(idxs_ap)
        _out_ap = g.lower_ap(out_ap)
        return g.add_instruction(
            mybir.InstDMAGatherAnt(
                name=nc.get_next_instruction_name(),
                ins=[*_in_ap, _idxs_ap, g.lower_val_access(g.to_reg(num_idxs))],
                outs=[_out_ap],
                transpose=False, num_idxs=num_idxs, elem_size=elem_size,
                stride_bytes_256=sb256, gen_mode=0, single_packet=False,
                queue_num=qn, sbuf_tokens_per_rank=0, sbuf_free_dim_per_rank=0,
                sbuf_free_dim_pad_per_rank=0, sbuf_byte_offset=0,
            )
        )

    with tile.TileContext(nc) as tc:
        with (
            tc.tile_pool(name="consts", bufs=1) as cpool,
            tc.tile_pool(name="mega", bufs=2) as rpool,
            tc.tile_pool(name="work", bufs=2) as wpool,
            tc.tile_pool(name="epil", bufs=2) as epool,
            tc.tile_pool(name="psum", bufs=2, space="PSUM") as pp,
            tc.tile_pool(name="dram", bufs=1, space="DRAM") as dpool,
        ):
            nc.gpsimd.load_library(library_config.mlp)
            # ---- constants
            w1aug_sb = cpool.tile([fin, W1ROW], f32, name="w1aug_sb")
            nc.sync.dma_start(w1aug_sb[:], w1aug)
            w2aug_sb = cpool.tile([h, W2ROW], f32, name="w2aug_sb")
            nc.sync.dma_start(w2aug_sb[:], w2aug)
            b1_sb = cpool.tile([h, 1], f32, name="b1_sb")
            nc.sync.dma_start(b1_sb[:], b1)
            b2rep_sb = cpool.tile([128, c], f32, name="b2rep_sb")
            nc.sync.dma_start(b2rep_sb[:], b2rep)
            e1_sb = cpool.tile([W1ROW, 1], f32, name="e1_sb")
            nc.sync.dma_start(e1_sb[:], e1)
            e2_sb = cpool.tile([W2ROW, 1], f32, name="e2_sb")
            nc.sync.dma_start(e2_sb[:], e2)
            ident = cpool.tile([128, 128], f32, name="ident")
            make_identity(nc, ident[:])
            ones_sb = cpool.tile([1, h], f32, name="ones_sb")
            nc.vector.memset(ones_sb[:], 1.0)
            iota_i = cpool.tile([128, SEGCAP], mybir.dt.int32, name="iota_i")
            nc.gpsimd.iota(iota_i[:], pattern=[[1, SEGCAP]], base=0,
                           channel_multiplier=0)
            iota_f = cpool.tile([128, SEGCAP], f32, name="iota_f")
            nc.vector.tensor_copy(iota_f[:], iota_i[:])
            segid_sb = cpool.tile([128, ncol], f32, name="segid_sb")
            nc.sync.dma_start(segid_sb[:], segid)
            scat_sb = cpool.tile([128, nsup * 32], i16, name="scat_sb")
            nc.sync.dma_start(scat_sb[:], scat16)
            adsl_sb = cpool.tile([128, nsup * 32], i16, name="adsl_sb")
            nc.sync.dma_start(adsl_sb[:], adsl16)
            onesM = cpool.tile([128, 128], f32, name="onesM")
            nc.vector.memset(onesM[:], 1.0)
            zt = cpool.tile([128, 2048], f32, name="zt")
            nc.vector.memset(zt[:], 0.0)

            # ---- internal DRAM tables
            h1s = dpool.tile([nshard, W1ROW], f32, name="h1s")
            h1f = dpool.tile([n, W1ROW], f32, name="h1f", addr_space="Shared")
            g2s = dpool.tile([npad, W2ROW], f32, name="g2s")
            g2f = dpool.tile([n, W2ROW], f32, name="g2f", addr_space="Shared")

            # zero-fill the scatter-add destinations
            for tbl in (g2s, out2):
                for o in range(0, npad, 4096):
                    nr = min(4096, npad - o)
                    nc.sync.dma_start(tbl[o:o + nr, :],
                                      zt[:, 0:nr * W2ROW // 128])

            # ---- phase 0: h1aug shard = (x @ W1aug) rows for this shard
            for nt in range(nshard // ntw):
                o = nt * ntw
                xt = epool.tile([fin, ntw], f32, name="xt")
                nc.sync.dma_start(xt[:], xT[:, o:o + ntw])
                psH = pp.tile([W1ROW, ntw], f32, name="psH", tag="pA")
                nc.tensor.matmul(psH[:], lhsT=w1aug_sb[:], rhs=xt[:],
                                 start=True, stop=True)
                h1t = epool.tile([W1ROW, ntw], f32, name="h1t")
                nc.scalar.activation(h1t[:], psH[:],
                                     mybir.ActivationFunctionType.Identity,
                                     bias=e1_sb[:])
                psT = pp.tile([qw, 4 * W1ROW], f32, name="psT", tag="pD")
                for q in range(4):
                    nc.tensor.transpose(
                        psT[:, q * W1ROW:(q + 1) * W1ROW],
                        in_=h1t[:, q * qw:(q + 1) * qw],
                        identity=ident[:],
                    )
                h1r = epool.tile([qw, 4 * W1ROW], f32, name="h1r")
                nc.vector.tensor_copy(h1r[:], psT[:])
                for q in range(4):
                    nc.sync.dma_start(
                        h1s[o + q * qw:o + (q + 1) * qw, :],
                        h1r[:, q * W1ROW:(q + 1) * W1ROW],
                    )

            nc.gpsimd.collective_compute(
                "AllGather", mybir.AluOpType.bypass,
                replica_groups=[list(range(N_CORES))],
                ins=[h1s[:]], outs=[h1f[:]],
            )

            # ---- edge phases
            def edge_layer(table, trow, adcol, adsb, fdim, out_dram, last):
                rw = fdim + 2
                tb256 = trow * 4 // 256
                for m in range(nmega):
                    ridx = rpool.tile([128, 1024], i16, name="ridx")
                    nc.sync.dma_start(
                        ridx[:], rowsidx[:, m * 1024:(m + 1) * 1024])
                    rows = rpool.tile([128, 128 * rw], f32, name="rows")
                    for k in range(NCHUNK):
                        lo = k * CHUNK
                        hi = min(lo + CHUNK, n)
                        for hh in range(2):
                            raw_dma_gather(
                                rows[:, (k * 32 + hh * 16) * rw:
                                     (k * 32 + hh * 16 + 16) * rw].rearrange(
                                    "p (b e) -> p b e", e=rw),
                                table[lo:hi, :],
                                ridx[:, k * 256 + hh * 128:
                                     k * 256 + (hh + 1) * 128],
                                2048, rw, tb256, qn=(k * 2 + hh) % 2)
                    # per-slot a_dst for the whole mega-group in one call
                    adcm = rpool.tile([128, 16], f32, name="adcm")
                    raw_dma_gather(
                        adcm[:].rearrange("p (b e) -> p b e", e=1),
                        adcol, adsl_sb[:, m * 128:(m + 1) * 128],
                        2048, 1, adsb)
                    rv = rows[:].rearrange("p (k s g e) -> p k s g e",
                                           k=NCHUNK, s=MEGA, e=rw)
                    for S_sub in range(MEGA):
                        S = m * MEGA + S_sub
                        adcyc = adcm[:, S_sub * 4:(S_sub + 1) * 4]
                        diag = wpool.tile([128, 4 * 128], f32, name="diag")
                        nc.vector.tensor_tensor(
                            out=diag[:].rearrange("p (q f) -> p q f", f=128),
                            in0=ident[:].unsqueeze(1).broadcast_to(
                                [128, 4, 128]),
                            in1=adcyc.unsqueeze(2).broadcast_to(
                                [128, 4, 128]),
                            op=mybir.AluOpType.mult)
                        psW = pp.tile([128, SUP * SEGCAP], f32, name="psW",
                                      tag="pB")
                        nc.tensor.matmul(psW[:], lhsT=onesM[:], rhs=diag[:],
                                         start=True, stop=True)
                        # unweighted one-hot
                        ohw = wpool.tile([128, 32 * SEGCAP], f32, name="oh")
                        ov = ohw[:].rearrange("p (c s) -> p c s", s=SEGCAP)
                        nc.vector.tensor_tensor(
                            out=ov,
                            in0=iota_f[:].unsqueeze(1).broadcast_to(
                                [128, 32, SEGCAP]),
                            in1=segid_sb[:, S * 32:(S + 1) * 32]
                                .unsqueeze(2).broadcast_to([128, 32, SEGCAP]),
                            op=mybir.AluOpType.is_equal)
                        # per-edge a_dst = reduce_s(ind * psW[slot])
                        scr = wpool.tile([128, 32 * SEGCAP], f32, name="scr")
                        psw_v = psW[:].rearrange(
                            "p (g s) -> p g s", s=SEGCAP).unsqueeze(
                            1).broadcast_to([128, NCHUNK, SUP, SEGCAP])
                        nc.vector.tensor_tensor(
                            out=scr[:].rearrange("p (k g s) -> p k g s",
                                                 k=NCHUNK, s=SEGCAP),
                            in0=ov.rearrange("p (k g) s -> p k g s",
                                             k=NCHUNK),
                            in1=psw_v,
                            op=mybir.AluOpType.mult)
                        es = wpool.tile([128, 32], f32, name="es")
                        nc.vector.tensor_reduce(
                            es[:].rearrange("p (k g) -> p k g", k=NCHUNK),
                            scr[:].rearrange("p (k g s) -> p k g s",
                                             k=NCHUNK, s=SEGCAP),
                            axis=mybir.AxisListType.X,
                            op=mybir.AluOpType.add)
                        # es += a_s ; lrelu ; exp
                        as_v = rv[:, :, S_sub][:, :, :, rw - 1]  # [128,4,8]
                        nc.vector.tensor_tensor(
                            out=es[:].rearrange("p (k g) -> p k g", k=NCHUNK),
                            in0=es[:].rearrange("p (k g) -> p k g", k=NCHUNK),
                            in1=as_v, op=mybir.AluOpType.add)
                        e2t = wpool.tile([128, 32], f32, name="e2")
                        nc.vector.tensor_scalar_mul(e2t[:], es[:], 0.2)
                        nc.vector.tensor_tensor(out=es[:], in0=es[:],
                                                in1=e2t[:],
                                                op=mybir.AluOpType.max)
                        ps = wpool.tile([128, 32], f32, name="ps")
                        nc.scalar.activation(ps[:], es[:],
                                             mybir.ActivationFunctionType.Exp)
                        nc.vector.tensor_tensor(
                            out=ov, in0=ov,
                            in1=ps[:].unsqueeze(2).broadcast_to(
                                [128, 32, SEGCAP]),
                            op=mybir.AluOpType.mult)
                        psA = pp.tile([rw, SUP * SEGCAP], f32,
                                      name="psA", tag="pA")
                        for g in range(SUP):
                            for k in range(NCHUNK):
                                off = ((k * MEGA + S_sub) * SUP + g) * rw
                                nc.tensor.matmul(
                                    psA[:, g * SEGCAP:(g + 1) * SEGCAP],
                                    lhsT=rows[:, off:off + rw],
                                    rhs=ohw[:, (k * SUP + g) * SEGCAP:
                                            (k * SUP + g + 1) * SEGCAP],
                                    start=(k == 0), stop=(k == NCHUNK - 1))
                        asb = epool.tile([rw, SUP * SEGCAP], f32,
                                         name="asb")
                        nc.vector.tensor_copy(asb[:], psA[:])
                        if not last:
                            denr = epool.tile([1, SUP * SEGCAP], f32,
                                              name="denr")
                            nc.vector.reciprocal(denr[:],
                                                 asb[fdim:fdim + 1, :])
                            psB = pp.tile([fdim, SUP * SEGCAP], f32,
                                          name="psB", tag="pB")
                            nc.tensor.matmul(psB[:], lhsT=ones_sb[:, 0:fdim],
                                             rhs=denr[:], start=True,
                                             stop=True)
                            hn = epool.tile([fdim, SUP * SEGCAP], f32,
                                            name="hn")
                            nc.vector.tensor_tensor(
                                out=hn[:], in0=asb[0:fdim, :], in1=psB[:],
                                op=mybir.AluOpType.mult)
                            h2r = epool.tile([fdim, SUP * SEGCAP], f32,
                                             name="h2r")
                            nc.scalar.activation(
                                h2r[:], hn[:],
                                mybir.ActivationFunctionType.Relu,
                                bias=b1_sb[:])
                            psC = pp.tile([W2ROW, SUP * SEGCAP], f32,
                                          name="psC", tag="pC")
                            nc.tensor.matmul(psC[:], lhsT=w2aug_sb[:],
                                             rhs=h2r[:], start=True, stop=True)
                            fin_t = epool.tile([W2ROW, SUP * SEGCAP], f32,
                                               name="fin1")
                            nc.scalar.activation(
                                fin_t[:], psC[:],
                                mybir.ActivationFunctionType.Identity,
                                bias=e2_sb[:])
                            ow = W2ROW
                            psD = pp.tile([128, 4 * ow], f32, name="psD",
                                          tag="pD")
                            for q in range(4):
                                nc.tensor.transpose(
                                    psD[:, q * ow:(q + 1) * ow],
                                    in_=fin_t[:, q * 128:(q + 1) * 128],
                                    identity=ident[0:ow, 0:ow])
                            orows = epool.tile([128, 4 * ow], f32,
                                               name="orows")
                            nc.vector.tensor_copy(orows[:], psD[:])
                            nc.gpsimd.dma_scatter_add(
                                out_dram[:, 0:ow],
                                orows[:].rearrange("p (q e) -> p q e", e=ow),
                                scat_sb[:, S * 32:(S + 1) * 32],
                                512, 512, ow)
                        else:
                            psD = pp.tile([128, 4 * rw], f32, name="psD",
                                          tag="pD")
                            for q in range(4):
                                nc.tensor.transpose(
                                    psD[:, q * rw:(q + 1) * rw],
                                    in_=asb[:, q * 128:(q + 1) * 128],
                                    identity=ident[0:rw, 0:rw])
                            oru = epool.tile([128, 4 * rw], f32, name="oru")
                            nc.vector.tensor_copy(oru[:], psD[:])
                            ouv = oru[:].rearrange("p (q e) -> p q e", e=rw)
                            rec = epool.tile([128, 4], f32, name="rec")
                            nc.vector.reciprocal(rec[:], ouv[:, :, fdim])
                            orows = epool.tile([128, 4 * c], f32,
                                               name="orows")
                            for q in range(4):
                                nc.vector.tensor_scalar_mul(
                                    orows[:, q * c:(q + 1) * c],
                                    oru[:, q * rw:q * rw + c],
                                    rec[:, q:q + 1])
                            nc.vector.tensor_tensor(
                                out=orows[:].rearrange(
                                    "p (q e) -> p q e", e=c),
                                in0=orows[:].rearrange(
                                    "p (q e) -> p q e", e=c),
                                in1=b2rep_sb[:].unsqueeze(1).broadcast_to(
                                    [128, 4, c]),
                                op=mybir.AluOpType.add)
                            nc.gpsimd.dma_scatter_add(
                                out_dram[:, 0:c],
                                orows[:].rearrange("p (q e) -> p q e", e=c),
                                scat_sb[:, S * 32:(S + 1) * 32],
                                512, 512, c, elem_step=W2ROW)

            edge_layer(h1f, W1ROW, h1s[:, h + 2:h + 3], 2, h, g2s, last=False)
            nc.gpsimd.collective_compute(
                "AllGather", mybir.AluOpType.bypass,
                replica_groups=[list(range(N_CORES))],
                ins=[g2s[0:nshard, :]], outs=[g2f[:]],
            )
            edge_layer(g2f, W2ROW, g2s[:, c + 2:c + 3], 1, c, out2, last=True)

    _compress_deps(nc)
    nc.compile()
    return nc


# ------------------------------------------------------------------ interface
def make_inmaps(inputs, cfg):
    x = np.ascontiguousarray(np.asarray(inputs["x"], np.float32))
    W1 = np.asarray(inputs["W1"], np.float32)
    as1 = np.asarray(inputs["att_src1"], np.float32)
    ad1 = np.asarray(inputs["att_dst1"], np.float32)
    b1 = np.asarray(inputs["b1"], np.float32)
    W2 = np.asarray(inputs["W2"], np.float32)
    as2 = np.asarray(inputs["att_src2"], np.float32)
    ad2 = np.asarray(inputs["att_dst2"], np.float32)
    b2 = np.asarray(inputs["b2"], np.float32)
    cores, nsup = preprocess(np.asarray(inputs["edge_index"]), cfg)
    h, cdim, fin = cfg["h"], cfg["c"], cfg["fin"]
    w1aug = np.zeros((fin, W1ROW), np.float32)
    w1aug[:, 0:h] = W1
    w1aug[:, h + 1] = W1 @ as1
    w1aug[:, h + 2] = W1 @ ad1
    w2aug = np.zeros((h, W2ROW), np.float32)
    w2aug[:, 0:cdim] = W2
    w2aug[:, cdim + 1] = W2 @ as2
    w2aug[:, cdim + 2] = W2 @ ad2
    e1v = np.zeros((W1ROW, 1), np.float32)
    e1v[h, 0] = 1.0
    e2v = np.zeros((W2ROW, 1), np.float32)
    e2v[cdim, 0] = 1.0
    nshard = cfg["nshard"]
    in_maps = []
    for cidx in range(N_CORES):
        xs = x[cidx * nshard:(cidx + 1) * nshard]
        in_maps.append(dict(
            xT=np.ascontiguousarray(xs.T),
            w1aug=w1aug, w2aug=w2aug,
            b1=np.ascontiguousarray(b1[:, None]),
            b2rep=np.ascontiguousarray(np.tile(b2[None, :], (128, 1))),
            e1=e1v, e2=e2v,
            rowsidx=cores[cidx]["rowsidx"],
            segid=cores[cidx]["segid"],
            scat16=cores[cidx]["scat16"],
            adsl16=cores[cidx]["adsl16"],
        ))
    return in_maps, nsup


def kernel(**inputs):
    from concourse import bass_utils

    cfg = dict(DEF_CFG)
    in_maps, nsup = make_inmaps(inputs, cfg)
    nc = build_program(cfg, nsup)
    res = bass_utils.run_bass_kernel_spmd(
        nc, in_maps, core_ids=list(range(N_CORES)))
    nshard = cfg["nshard"]
    out = np.concatenate(
        [res.results[c]["out2"][:nshard, :cfg["c"]] for c in range(N_CORES)], 0)
    return out.astype(np.float32)


# revision 42
# speedup vs baseline: 1.1753x; 1.0210x over previous
"""Two-layer single-head GAT (PyG GATConv semantics) on 8 Trainium2 NeuronCores.

Strategy (dst-sharded edge-parallel, dma_gather-based):
  * Host: add self-loops, sort edges by destination, shard destinations
    across the 8 cores (12500 each).  Segments (per-dst edge runs) are packed
    into GROUPS of <= 64 consecutive segments such that the group's edges
    from each 32768-row source-table window ("chunk") number <= 128.  Each
    group occupies 4 chunk-pure columns of 128 edge slots (one per chunk);
    8 groups form a super-tile (512 psum slots); 4 super-tiles form a
    mega-group whose per-edge source rows are fetched with FOUR
    InstDMAGatherAnt calls (one per table window, int16 indices, 4096 rows
    each) and whose per-edge destination a_dst scalars are fetched with ONE
    more (elem_size=1, indices are shard-local).  This replaces thousands of
    one-index-per-partition SWDGE indirect DMAs (the 10.7ms baseline
    bottleneck: ~1us Q7 descriptor-emission fixed cost per call).
  * Tables are padded to power-of-two row strides (128/64 f32) so row
    addresses encode in dma_gather's stride_bytes_256.  Table rows are
    [feat | 1 | a_src | a_dst | 0...]; gathering feat+2 elements brings the
    constant-1 column that makes the aggregation matmul emit the softmax
    denominator directly.
  * Per super-tile: attention logits e = leakyrelu(a_s + a_d) and p=exp(e)
    in 4 whole-super-tile ops; the exp weights fold into the segment one-hot
    (built in 2 broadcast tensor_tensor ops); 32 matmuls accumulate
    psA[rw, 512] (4 chunk-columns per group); normalisation (+relu+W2 for
    layer 1), transposes, and ONE dma_scatter_add write the per-destination
    rows into the pre-zeroed padded output table (pad slots hit a dump row).
  * AllGather replicates the layer tables between phases.
"""

import numpy as np

N_NODES = 100000
N_CORES = 8
F_IN, H, C = 128, 64, 40

CHUNK = 32768          # dma_gather int16 index window (table rows per window)
NCHUNK = 4             # ceil(100000 / 32768)
GCOL = 128             # edge slots per (group, chunk) column
SEGCAP = 64            # segments per group
SUP = 8                # groups per super-tile  -> 512 psum slots
MEGA = 4               # super-tiles per gather mega-group
DUMMY_SEG = 99.0
W1ROW = 128            # padded layer-1 table row (f32): 512B stride
W2ROW = 64             # padded layer-2 table row (f32): 256B stride

DEF_CFG = dict(
    n=N_NODES, nshard=N_NODES // N_CORES, fin=F_IN, h=H, c=C, ntw=500,
)


# ----------------------------------------------------------------- host prep
def _wrap16(entries):
    """[n] -> [128, n/16] int16: entry i at [i%16, i//16], replicated x8."""
    n = entries.shape[0]
    assert n % 16 == 0
    w = entries.reshape(n // 16, 16).T.astype(np.int16)  # [16, n/16]
    return np.tile(w, (8, 1))


def _pack_core(src_c, dst_c, base, nshard):
    """Group dst-sorted edges: per group, <=SEGCAP consecutive segments with
    <=GCOL edges per source chunk.  Returns per-group data."""
    counts = np.bincount(dst_c - base, minlength=nshard)
    assert counts.min() >= 1
    cum = np.concatenate([[0], np.cumsum(counts)])
    chunk_of = (src_c // CHUNK).astype(np.int64)
    # per-segment chunk counts [nshard, 4]
    segck = np.zeros((nshard, NCHUNK), np.int64)
    for k in range(NCHUNK):
        np.add.at(segck[:, k], dst_c[chunk_of == k] - base, 1)
    assert segck.max() <= GCOL, "single segment overflows a chunk column"
    groups = []
    i = 0
    while i < nshard:
        acc = np.zeros(NCHUNK, np.int64)
        j = i
        while j < nshard and j - i < SEGCAP and (acc + segck[j]).max() <= GCOL:
            acc += segck[j]
            j += 1
        groups.append((i, j))
        i = j
    return groups, cum, chunk_of


def preprocess(edge_index, cfg):
    n, nshard = cfg["n"], cfg["nshard"]
    src = np.asarray(edge_index[0]).astype(np.int64)
    dst = np.asarray(edge_index[1]).astype(np.int64)
    loop = np.arange(n, dtype=np.int64)
    src = np.concatenate([src, loop])
    dst = np.concatenate([dst, loop])
    order = np.argsort(dst, kind="stable")
    src, dst = src[order], dst[order]
    bounds = np.searchsorted(dst, np.arange(N_CORES + 1) * nshard)
    packed = []
    ngmax = 0
    for cc in range(N_CORES):
        s, d = src[bounds[cc]:bounds[cc + 1]], dst[bounds[cc]:bounds[cc + 1]]
        groups, cum, chunk_of = _pack_core(s, d, cc * nshard, nshard)
        packed.append((s, d, groups, cum, chunk_of))
        ngmax = max(ngmax, len(groups))
    nsup = -(-ngmax // SUP)
    nmega = -(-nsup // MEGA)
    nsup = nmega * MEGA
    ng = nsup * SUP
    ncol = nsup * 32  # columns per layer (kappa*8+g per super-tile)

    cores = []
    for cc in range(N_CORES):
        s, d, groups, cum, chunk_of = packed[cc]
        base = cc * nshard
        # per-column edge lists
        rowsidx = np.zeros((nmega, NCHUNK, MEGA, SUP, GCOL), np.int64)
        segid = np.full((128, ncol), DUMMY_SEG, np.float32)
        scat = np.full((nsup, SUP * SEGCAP), nshard, np.int64)
        for gi in range(len(groups)):
            i, j = groups[gi]
            S, g = gi // SUP, gi % SUP
            m, S_sub = S // MEGA, S % MEGA
            e0, e1 = int(cum[i]), int(cum[j])
            ck = chunk_of[e0:e1]
            sg = s[e0:e1]
            dg = d[e0:e1]
            for k in range(NCHUNK):
                sel = np.where(ck == k)[0]
                cnt = sel.shape[0]
                assert cnt <= GCOL
                rowsidx[m, k, S_sub, g, :cnt] = sg[sel] - k * CHUNK
                col = S * 32 + k * SUP + g
                segid[:cnt, col] = (dg[sel] - base - i).astype(np.float32)
            scat[S, g * SEGCAP: g * SEGCAP + (j - i)] = np.arange(i, j)
        adsl = np.minimum(scat, nshard - 1)
        cores.append(dict(
            rowsidx=_wrap16(rowsidx.reshape(-1)).reshape(128, -1),
            segid=segid,
            scat16=np.concatenate(
                [_wrap16(scat[S]) for S in range(nsup)], axis=1),
            adsl16=np.concatenate(
                [_wrap16(adsl[S]) for S in range(nsup)], axis=1),
        ))
    return cores, nsup


def _compress_deps(nc):
    """Drop redundant sync dependencies so walrus' per-instruction HW wait
    slots don't overflow.  Producers on the same engine execute in issue
    order, and DMAs on the same logical queue complete in FIFO order, so a
    dependency on the latest producer of each stream subsumes the earlier
    ones.  Collectives are never dropped."""
    f = nc.m.functions[0]

    def all_insts(blk):
        for i in blk.instructions:
            yield i
        for sb in getattr(blk, "blocks", []) or []:
            yield from all_insts(sb)

    insts = [i for b in f.blocks for i in all_insts(b)]
    pos = {i.name: p for p, i in enumerate(insts)}
    by_name = {i.name: i for i in insts}

    def stream_key(p):
        tname = type(p).__name__
        if tname == "InstCollectiveCompute":
            return None  # own completion semaphore; never compress
        # completion order is FIFO only within one scheduled proc lane
        # (engine proc, or a DMAHW/DMASW semaphore lane)
        proc = getattr(p, "bass_scheduled_proc", None)
        if proc is None:
            return None
        return ("proc", proc)

    for i in insts:
        deps = list(i.sync_dependency_names())
        if len(deps) <= 2:
            continue
        best: dict = {}
        keep = []
        for d in deps:
            p = by_name.get(d)
            if p is None:
                keep.append(d)
                continue
            k = stream_key(p)
            if k is None:
                keep.append(d)
                continue
            cur = best.get(k)
            if cur is None or pos[d] > pos[cur]:
                best[k] = d
        keep += list(best.values())
        for d in deps:
            if d not in keep:
                i.try_remove_dependency(d)


# ------------------------------------------------------------- device program
def build_program(cfg, nsup, debug=False):
    import concourse.bass as bass
    import concourse.bacc as bacc
    import concourse.mybir as mybir
    import concourse.tile as tile
    from concourse import library_config
    from concourse.masks import make_identity

    f32 = mybir.dt.float32
    i16 = mybir.dt.int16
    nshard, fin, h, c = cfg["nshard"], cfg["fin"], cfg["h"], cfg["c"]
    ntw = cfg["ntw"]
    n = cfg["n"]
    nmega = nsup // MEGA
    ncol = nsup * 32
    qw = ntw // 4
    npad = nshard + 128  # padded tables: dump rows at [nshard, npad)

    nc = bacc.Bacc(
        "TRN2", target_bir_lowering=False, debug=False,
        enable_asserts=False, num_devices=N_CORES,
        dynamic_dma_scratch_size=36864, num_swdge_queues=4,
    )

    xT = nc.dram_tensor("xT", [fin, nshard], f32, kind="ExternalInput").ap()
    w1aug = nc.dram_tensor("w1aug", [fin, W1ROW], f32, kind="ExternalInput").ap()
    w2aug = nc.dram_tensor("w2aug", [h, W2ROW], f32, kind="ExternalInput").ap()
    b1 = nc.dram_tensor("b1", [h, 1], f32, kind="ExternalInput").ap()
    b2rep = nc.dram_tensor("b2rep", [128, c], f32, kind="ExternalInput").ap()
    e1 = nc.dram_tensor("e1", [W1ROW, 1], f32, kind="ExternalInput").ap()
    e2 = nc.dram_tensor("e2", [W2ROW, 1], f32, kind="ExternalInput").ap()
    rowsidx = nc.dram_tensor("rowsidx", [128, nmega * 1024], i16,
                             kind="ExternalInput").ap()
    segid = nc.dram_tensor("segid", [128, ncol], f32, kind="ExternalInput").ap()
    scat16 = nc.dram_tensor("scat16", [128, nsup * 32], i16,
                            kind="ExternalInput").ap()
    adsl16 = nc.dram_tensor("adsl16", [128, nsup * 32], i16,
                            kind="ExternalInput").ap()
    out2 = nc.dram_tensor("out2", [npad, W2ROW], f32, kind="ExternalOutput").ap()

    def raw_dma_gather(out_ap, in_ap, idxs_ap, num_idxs, elem_size, sb256,
                       qn=0):
        # <=1024 idxs per call: 65 descriptors fits the SWDGE ring with room
        # to pipeline, and the 64-data-desc packet stays within the SDMA
        # packet limit (single_packet keeps the ~35ns/desc drain rate).
        g = nc.gpsimd
        _in_ap = g.lower_ap_dma(in_ap, for_custom_bir_dma=True)
        _idxs_ap = g.lower_ap(idxs_ap)
        _out_ap = g.lower_ap(out_ap)
        return g.add_instruction(
            mybir.InstDMAGatherAnt(
                name=nc.get_next_instruction_name(),
                ins=[*_in_ap, _idxs_ap, g.lower_val_access(g.to_reg(num_idxs))],
                outs=[_out_ap],
                transpose=False, num_idxs=num_idxs, elem_size=elem_size,
                stride_bytes_256=sb256, gen_mode=0, single_packet=False,
                queue_num=qn, sbuf_tokens_per_rank=0, sbuf_free_dim_per_rank=0,
                sbuf_free_dim_pad_per_rank=0, sbuf_byte_offset=0,
            )
        )

    with tile.TileContext(nc) as tc:
        with (
            tc.tile_pool(name="consts", bufs=1) as cpool,
            tc.tile_pool(name="mega", bufs=2) as rpool,
            tc.tile_pool(name="work", bufs=2) as wpool,
            tc.tile_pool(name="epil", bufs=2) as epool,
            tc.tile_pool(name="psum", bufs=2, space="PSUM") as pp,
            tc.tile_pool(name="dram", bufs=1, space="DRAM") as dpool,
        ):
            nc.gpsimd.load_library(library_config.mlp)
            # ---- constants
            w1aug_sb = cpool.tile([fin, W1ROW], f32, name="w1aug_sb")
            nc.sync.dma_start(w1aug_sb[:], w1aug)
            w2aug_sb = cpool.tile([h, W2ROW], f32, name="w2aug_sb")
            nc.sync.dma_start(w2aug_sb[:], w2aug)
            b1_sb = cpool.tile([h, 1], f32, name="b1_sb")
            nc.sync.dma_start(b1_sb[:], b1)
            b2rep_sb = cpool.tile([128, c], f32, name="b2rep_sb")
            nc.sync.dma_start(b2rep_sb[:], b2rep)
            e1_sb = cpool.tile([W1ROW, 1], f32, name="e1_sb")
            nc.sync.dma_start(e1_sb[:], e1)
            e2_sb = cpool.tile([W2ROW, 1], f32, name="e2_sb")
            nc.sync.dma_start(e2_sb[:], e2)
            ident = cpool.tile([128, 128], f32, name="ident")
            make_identity(nc, ident[:])
            ones_sb = cpool.tile([1, h], f32, name="ones_sb")
            nc.vector.memset(ones_sb[:], 1.0)
            iota_i = cpool.tile([128, SEGCAP], mybir.dt.int32, name="iota_i")
            nc.gpsimd.iota(iota_i[:], pattern=[[1, SEGCAP]], base=0,
                           channel_multiplier=0)
            iota_f = cpool.tile([128, SEGCAP], f32, name="iota_f")
            nc.vector.tensor_copy(iota_f[:], iota_i[:])
            segid_sb = cpool.tile([128, ncol], f32, name="segid_sb")
            nc.sync.dma_start(segid_sb[:], segid)
            scat_sb = cpool.tile([128, nsup * 32], i16, name="scat_sb")
            nc.sync.dma_start(scat_sb[:], scat16)
            adsl_sb = cpool.tile([128, nsup * 32], i16, name="adsl_sb")
            nc.sync.dma_start(adsl_sb[:], adsl16)
            onesM = cpool.tile([128, 128], f32, name="onesM")
            nc.vector.memset(onesM[:], 1.0)
            zt = cpool.tile([128, 2048], f32, name="zt")
            nc.vector.memset(zt[:], 0.0)

            # ---- internal DRAM tables
            h1s = dpool.tile([nshard, W1ROW], f32, name="h1s")
            h1f = dpool.tile([n, W1ROW], f32, name="h1f", addr_space="Shared")
            g2s = dpool.tile([npad, W2ROW], f32, name="g2s")
            g2f = dpool.tile([n, W2ROW], f32, name="g2f", addr_space="Shared")

            # zero-fill the scatter-add destinations
            for tbl in (g2s, out2):
                for o in range(0, npad, 4096):
                    nr = min(4096, npad - o)
                    nc.sync.dma_start(tbl[o:o + nr, :],
                                      zt[:, 0:nr * W2ROW // 128])

            # ---- phase 0: h1aug shard = (x @ W1aug) rows for this shard
            for nt in range(nshard // ntw):
                o = nt * ntw
                xt = epool.tile([fin, ntw], f32, name="xt")
                nc.sync.dma_start(xt[:], xT[:, o:o + ntw])
                psH = pp.tile([W1ROW, ntw], f32, name="psH", tag="pA")
                nc.tensor.matmul(psH[:], lhsT=w1aug_sb[:], rhs=xt[:],
                                 start=True, stop=True)
                h1t = epool.tile([W1ROW, ntw], f32, name="h1t")
                nc.scalar.activation(h1t[:], psH[:],
                                     mybir.ActivationFunctionType.Identity,
                                     bias=e1_sb[:])
                psT = pp.tile([qw, 4 * W1ROW], f32, name="psT", tag="pD")
                for q in range(4):
                    nc.tensor.transpose(
                        psT[:, q * W1ROW:(q + 1) * W1ROW],
                        in_=h1t[:, q * qw:(q + 1) * qw],
                        identity=ident[:],
                    )
                h1r = epool.tile([qw, 4 * W1ROW], f32, name="h1r")
                nc.vector.tensor_copy(h1r[:], psT[:])
                for q in range(4):
                    nc.sync.dma_start(
                        h1s[o + q * qw:o + (q + 1) * qw, :],
                        h1r[:, q * W1ROW:(q + 1) * W1ROW],
                    )

            nc.gpsimd.collective_compute(
                "AllGather", mybir.AluOpType.bypass,
                replica_groups=[list(range(N_CORES))],
                ins=[h1s[:]], outs=[h1f[:]],
            )

            # ---- edge phases
            def edge_layer(table, trow, adcol, adsb, fdim, out_dram, last):
                rw = fdim + 2
                tb256 = trow * 4 // 256
                for m in range(nmega):
                    ridx = rpool.tile([128, 1024], i16, name="ridx")
                    nc.sync.dma_start(
                        ridx[:], rowsidx[:, m * 1024:(m + 1) * 1024])
                    rows = rpool.tile([128, 128 * rw], f32, name="rows")
                    for k in range(NCHUNK):
                        lo = k * CHUNK
                        hi = min(lo + CHUNK, n)
                        for hh in range(4):
                            raw_dma_gather(
                                rows[:, (k * 32 + hh * 8) * rw:
                                     (k * 32 + hh * 8 + 8) * rw].rearrange(
                                    "p (b e) -> p b e", e=rw),
                                table[lo:hi, :],
                                ridx[:, k * 256 + hh * 64:
                                     k * 256 + (hh + 1) * 64],
                                1024, rw, tb256, qn=(k * 4 + hh) % 4)
                    # per-slot a_dst for the whole mega-group in one call
                    adcm = rpool.tile([128, 16], f32, name="adcm")
                    for hh in range(2):
                        raw_dma_gather(
                            adcm[:, hh * 8:(hh + 1) * 8].rearrange(
                                "p (b e) -> p b e", e=1),
                            adcol,
                            adsl_sb[:, m * 128 + hh * 64:
                                    m * 128 + (hh + 1) * 64],
                            1024, 1, adsb, qn=2 + hh)
                    rv = rows[:].rearrange("p (k s g e) -> p k s g e",
                                           k=NCHUNK, s=MEGA, e=rw)
                    for S_sub in range(MEGA):
                        S = m * MEGA + S_sub
                        adcyc = adcm[:, S_sub * 4:(S_sub + 1) * 4]
                        diag = wpool.tile([128, 4 * 128], f32, name="diag")
                        nc.vector.tensor_tensor(
                            out=diag[:].rearrange("p (q f) -> p q f", f=128),
                            in0=ident[:].unsqueeze(1).broadcast_to(
                                [128, 4, 128]),
                            in1=adcyc.unsqueeze(2).broadcast_to(
                                [128, 4, 128]),
                            op=mybir.AluOpType.mult)
                        psW = pp.tile([128, SUP * SEGCAP], f32, name="psW",
                                      tag="pB")
                        nc.tensor.matmul(psW[:], lhsT=onesM[:], rhs=diag[:],
                                         start=True, stop=True)
                        # unweighted one-hot
                        ohw = wpool.tile([128, 32 * SEGCAP], f32, name="oh")
                        ov = ohw[:].rearrange("p (c s) -> p c s", s=SEGCAP)
                        nc.vector.tensor_tensor(
                            out=ov,
                            in0=iota_f[:].unsqueeze(1).broadcast_to(
                                [128, 32, SEGCAP]),
                            in1=segid_sb[:, S * 32:(S + 1) * 32]
                                .unsqueeze(2).broadcast_to([128, 32, SEGCAP]),
                            op=mybir.AluOpType.is_equal)
                        # per-edge a_dst = reduce_s(ind * psW[slot])
                        scr = wpool.tile([128, 32 * SEGCAP], f32, name="scr")
                        psw_v = psW[:].rearrange(
                            "p (g s) -> p g s", s=SEGCAP).unsqueeze(
                            1).broadcast_to([128, NCHUNK, SUP, SEGCAP])
                        nc.vector.tensor_tensor(
                            out=scr[:].rearrange("p (k g s) -> p k g s",
                                                 k=NCHUNK, s=SEGCAP),
                            in0=ov.rearrange("p (k g) s -> p k g s",
                                             k=NCHUNK),
                            in1=psw_v,
                            op=mybir.AluOpType.mult)
                        es = wpool.tile([128, 32], f32, name="es")
                        nc.vector.tensor_reduce(
                            es[:].rearrange("p (k g) -> p k g", k=NCHUNK),
                            scr[:].rearrange("p (k g s) -> p k g s",
                                             k=NCHUNK, s=SEGCAP),
                            axis=mybir.AxisListType.X,
                            op=mybir.AluOpType.add)
                        # es += a_s ; lrelu ; exp
                        as_v = rv[:, :, S_sub][:, :, :, rw - 1]  # [128,4,8]
                        nc.vector.tensor_tensor(
                            out=es[:].rearrange("p (k g) -> p k g", k=NCHUNK),
                            in0=es[:].rearrange("p (k g) -> p k g", k=NCHUNK),
                            in1=as_v, op=mybir.AluOpType.add)
                        e2t = wpool.tile([128, 32], f32, name="e2")
                        nc.vector.tensor_scalar_mul(e2t[:], es[:], 0.2)
                        nc.vector.tensor_tensor(out=es[:], in0=es[:],
                                                in1=e2t[:],
                                                op=mybir.AluOpType.max)
                        ps = wpool.tile([128, 32], f32, name="ps")
                        nc.scalar.activation(ps[:], es[:],
                                             mybir.ActivationFunctionType.Exp)
                        nc.vector.tensor_tensor(
                            out=ov, in0=ov,
                            in1=ps[:].unsqueeze(2).broadcast_to(
                                [128, 32, SEGCAP]),
                            op=mybir.AluOpType.mult)
                        psA = pp.tile([rw, SUP * SEGCAP], f32,
                                      name="psA", tag="pA")
                        for g in range(SUP):
                            for k in range(NCHUNK):
                                off = ((k * MEGA + S_sub) * SUP + g) * rw
                                nc.tensor.matmul(
                                    psA[:, g * SEGCAP:(g + 1) * SEGCAP],
                                    lhsT=rows[:, off:off + rw],
                                    rhs=ohw[:, (k * SUP + g) * SEGCAP:
                                            (k * SUP + g + 1) * SEGCAP],
                                    start=(k == 0), stop=(k == NCHUNK - 1))
                        asb = epool.tile([rw, SUP * SEGCAP], f32,
                                         name="asb")
                        nc.vector.tensor_copy(asb[:], psA[:])
                        if not last:
                            denr = epool.tile([1, SUP * SEGCAP], f32,
                                              name="denr")
                            nc.vector.reciprocal(denr[:],
                                                 asb[fdim:fdim + 1, :])
                            psB = pp.tile([fdim, SUP * SEGCAP], f32,
                                          name="psB", tag="pB")
                            nc.tensor.matmul(psB[:], lhsT=ones_sb[:, 0:fdim],
                                             rhs=denr[:], start=True,
                                             stop=True)
                            hn = epool.tile([fdim, SUP * SEGCAP], f32,
                                            name="hn")
                            nc.vector.tensor_tensor(
                                out=hn[:], in0=asb[0:fdim, :], in1=psB[:],
                                op=mybir.AluOpType.mult)
                            h2r = epool.tile([fdim, SUP * SEGCAP], f32,
                                             name="h2r")
                            nc.scalar.activation(
                                h2r[:], hn[:],
                                mybir.ActivationFunctionType.Relu,
                                bias=b1_sb[:])
                            psC = pp.tile([W2ROW, SUP * SEGCAP], f32,
                                          name="psC", tag="pC")
                            nc.tensor.matmul(psC[:], lhsT=w2aug_sb[:],
                                             rhs=h2r[:], start=True, stop=True)
                            fin_t = epool.tile([W2ROW, SUP * SEGCAP], f32,
                                               name="fin1")
                            nc.scalar.activation(
                                fin_t[:], psC[:],
                                mybir.ActivationFunctionType.Identity,
                                bias=e2_sb[:])
                            ow = W2ROW
                            psD = pp.tile([128, 4 * ow], f32, name="psD",
                                          tag="pD")
                            for q in range(4):
                                nc.tensor.transpose(
                                    psD[:, q * ow:(q + 1) * ow],
                                    in_=fin_t[:, q * 128:(q + 1) * 128],
                                    identity=ident[0:ow, 0:ow])
                            orows = epool.tile([128, 4 * ow], f32,
                                               name="orows")
                            nc.vector.tensor_copy(orows[:], psD[:])
                            nc.gpsimd.dma_scatter_add(
                                out_dram[:, 0:ow],
                                orows[:].rearrange("p (q e) -> p q e", e=ow),
                                scat_sb[:, S * 32:(S + 1) * 32],
                                512, 512, ow)
                        else:
                            psD = pp.tile([128, 4 * rw], f32, name="psD",
                                          tag="pD")
                            for q in range(4):
                                nc.tensor.transpose(
                                    psD[:, q * rw:(q + 1) * rw],
                                    in_=asb[:, q * 128:(q + 1) * 128],
                                    identity=ident[0:rw, 0:rw])
                            oru = epool.tile([128, 4 * rw], f32, name="oru")
                            nc.vector.tensor_copy(oru[:], psD[:])
                            ouv = oru[:].rearrange("p (q e) -> p q e", e=rw)
                            rec = epool.tile([128, 4], f32, name="rec")
                            nc.vector.reciprocal(rec[:], ouv[:, :, fdim])
                            orows = epool.tile([128, 4 * c], f32,
                                               name="orows")
                            for q in range(4):
                                nc.vector.tensor_scalar_mul(
                                    orows[:, q * c:(q + 1) * c],
                                    oru[:, q * rw:q * rw + c],
                                    rec[:, q:q + 1])
                            nc.vector.tensor_tensor(
                                out=orows[:].rearrange(
                                    "p (q e) -> p q e", e=c),
                                in0=orows[:].rearrange(
                                    "p (q e) -> p q e", e=c),
                                in1=b2rep_sb[:].unsqueeze(1).broadcast_to(
                                    [128, 4, c]),
                                op=mybir.AluOpType.add)
                            nc.gpsimd.dma_scatter_add(
                                out_dram[:, 0:c],
                                orows[:].rearrange("p (q e) -> p q e", e=c),
                                scat_sb[:, S * 32:(S + 1) * 32],
                                512, 512, c, elem_step=W2ROW)

            edge_layer(h1f, W1ROW, h1s[:, h + 2:h + 3], 2, h, g2s, last=False)
            nc.gpsimd.collective_compute(
                "AllGather", mybir.AluOpType.bypass,
                replica_groups=[list(range(N_CORES))],
                ins=[g2s[0:nshard, :]], outs=[g2f[:]],
            )
            edge_layer(g2f, W2ROW, g2s[:, c + 2:c + 3], 1, c, out2, last=True)

    _compress_deps(nc)
    nc.compile()
    return nc


# ------------------------------------------------------------------ interface
def make_inmaps(inputs, cfg):
    x = np.ascontiguousarray(np.asarray(inputs["x"], np.float32))
    W1 = np.asarray(inputs["W1"], np.float32)
    as1 = np.asarray(inputs["att_src1"], np.float32)
    ad1 = np.asarray(inputs["att_dst1"], np.float32)
    b1 = np.asarray(inputs["b1"], np.float32)
    W2 = np.asarray(inputs["W2"], np.float32)
    as2 = np.asarray(inputs["att_src2"], np.float32)
    ad2 = np.asarray(inputs["att_dst2"], np.float32)
    b2 = np.asarray(inputs["b2"], np.float32)
    cores, nsup = preprocess(np.asarray(inputs["edge_index"]), cfg)
    h, cdim, fin = cfg["h"], cfg["c"], cfg["fin"]
    w1aug = np.zeros((fin, W1ROW), np.float32)
    w1aug[:, 0:h] = W1
    w1aug[:, h + 1] = W1 @ as1
    w1aug[:, h + 2] = W1 @ ad1
    w2aug = np.zeros((h, W2ROW), np.float32)
    w2aug[:, 0:cdim] = W2
    w2aug[:, cdim + 1] = W2 @ as2
    w2aug[:, cdim + 2] = W2 @ ad2
    e1v = np.zeros((W1ROW, 1), np.float32)
    e1v[h, 0] = 1.0
    e2v = np.zeros((W2ROW, 1), np.float32)
    e2v[cdim, 0] = 1.0
    nshard = cfg["nshard"]
    in_maps = []
    for cidx in range(N_CORES):
        xs = x[cidx * nshard:(cidx + 1) * nshard]
        in_maps.append(dict(
            xT=np.ascontiguousarray(xs.T),
            w1aug=w1aug, w2aug=w2aug,
            b1=np.ascontiguousarray(b1[:, None]),
            b2rep=np.ascontiguousarray(np.tile(b2[None, :], (128, 1))),
            e1=e1v, e2=e2v,
            rowsidx=cores[cidx]["rowsidx"],
            segid=cores[cidx]["segid"],
            scat16=cores[cidx]["scat16"],
            adsl16=cores[cidx]["adsl16"],
        ))
    return in_maps, nsup


def kernel(**inputs):
    from concourse import bass_utils

    cfg = dict(DEF_CFG)
    in_maps, nsup = make_inmaps(inputs, cfg)
    nc = build_program(cfg, nsup)
    res = bass_utils.run_bass_kernel_spmd(
        nc, in_maps, core_ids=list(range(N_CORES)))
    nshard = cfg["nshard"]
    out = np.concatenate(
        [res.results[c]["out2"][:nshard, :cfg["c"]] for c in range(N_CORES)], 0)
    return out.astype(np.float32)
